# revision 10
# baseline (speedup 1.0000x reference)
"""Trainium2 Bass kernel for a WaveNet-style dilated-conv stack (v4).

Network (per reference):
  x1 = conv1d(x, Wc, bc, d=1, pad=1)                      # 1 -> 32, host-side
  for l in 27 layers, d = 2^(l%9):
      g = tanh(conv(x, Wt_l, d)) * sigmoid(conv(x, Ws_l, d))   # 32->32, k=3, pad=d
      skip += conv1x1(g, Wskip_l)                              # 32->512
      x = conv1x1(g, Wdense_l) + x
  out = conv1x1(relu(conv1x1(skip, Wp1)), Wp2)            # 512->512->256
  return log_softmax(out, axis=channels)

v4 design (8 cores, sequence-parallel, no cross-core comm):
  - Wp1 folded into skip weights (W1s = Wp1 @ Wskip) on host.
  - Per core 2 halves of 8192 cols; per-layer SHRINKING windows: layer l
    computes radius R_l = 512*ceil(S_{l+1}/512) (S = suffix dilation sum),
    R = 1536/1024/512/0.  Strip grid anchored at PAD; units are 4-strip
    aligned (8-strip super units + small boundary units).
  - Gated convs: one K=96 matmul per (strip, fn), col-tiled 4-way by
    sigma%4 into [128, 1024] f32 psum per fn; tanh/sigmoid at N=1024.
  - g stored stacked [128, 1024] bf16; ring copies to 7 per-group ring
    tiles [128, 8192] in a PERMUTED valid-strip order so each copy is one
    contiguous [32, <=1024] 4x-mode DVE op.  Rings for all 27 layers live
    simultaneously -> NO h accumulator, single skip sweep at end of half.
  - Dense conv: ONE stacked matmul (block-diag lhsT, M=128, K=32*strips)
    per 2048-block; evac+residual per strip: rows 1,3 -> DVE stt (psum
    cross-partition ok), rows 0,2 -> ScalarE Identity+bias to a [64,512]
    tmp at rows 32:64 + GpSimd tensor_add into xx (all same-partition).
  - Tap replicas (x>>d, x<<d on partition strips 0:32 / 64:96) maintained
    by 2 SBUF->SBUF DMAs per unit on the Sync queue.
  - Skip+post sweep per half: per 4-cb group, per m: 7 K-chunk matmuls
    (K=96 for the last 3-layer group: avoids reading unwritten ring rows),
    cb-inner for LDWEIGHTS reuse; DVE relu(x+hb) evac to rr.  Post per cb:
    8 Wp2 matmuls -> pos [128,1024] psum, Exp/Identity acts (bias bp2),
    sum-broadcast via 2 accumulating ones-matmuls, chunked Ln over 8 cbs
    (exp/ln stay in one table set per chunk -> no ACT_TABLE thrash), final
    log_softmax subtract on DVE, bf16 output DMA (host casts to f32).
"""

import numpy as np
import ml_dtypes

BF16 = ml_dtypes.bfloat16

DIL = [2 ** i for i in range(9)] * 3
L = len(DIL)            # 27
RD, SD, QD = 32, 512, 256
T = 131072
NCORES = 8
V = T // NCORES         # 16384 per core
VH = V // 2             # 8192 per half
HALO = 1536
PAD = 256
WH = VH + 2 * HALO      # 11264 max computed window per half
WA = WH + 2 * PAD       # 11776 allocated width per half
VOFF = HALO + PAD       # 1792 valid-region offset
NB = VH // 512          # 16 valid 512-col strips per half
NGRP = (L + 3) // 4     # 7 groups of (up to) 4 layers

# per-layer computed radius (cols), 512-aligned; S_{l+1} = sum(DIL[l+1:])
_S = [sum(DIL[i + 1:]) for i in range(L)]
R_L = [512 * ((s + 511) // 512) for s in _S]      # 1536x8, 1024x9, 512x9, 0
for _l in range(L):
    assert R_L[_l] >= _S[_l] and R_L[_l] <= HALO

# valid strips: global strip sigma covers flat cols [PAD+512s, PAD+512s+512)
SV0 = (VOFF - PAD) // 512        # 3 = first valid strip
SV1 = SV0 + NB - 1               # 18 = last valid strip


def _units_for(R):
    """Unit list [(sigma_start, n_strips)] for radius R; interior units are
    4-strip aligned (up to 8 strips), boundary-left unit stays within one
    4-strip block."""
    s0 = SV0 - R // 512
    s1 = SV1 + 1 + R // 512
    units = []
    sa = s0
    if s0 % 4:
        ns = min(4 - s0 % 4, s1 - s0)
        units.append((s0, ns))
        sa = s0 + ns
    while sa < s1:
        ns = min(8, s1 - sa)
        units.append((sa, ns))
        sa += ns
    return units


UNITS_L = [_units_for(R) for R in R_L]


def _ring_pos(sigma):
    """Permuted ring column position for valid strip sigma (see docstring)."""
    r = sigma % 4
    b = sigma // 4
    bmin = 1 if r < 3 else 0
    return r * 4 + (b - bmin)


_cache = {}
_last_run = {}


def _build():
    from contextlib import ExitStack

    import concourse.bacc as bacc
    import concourse.mybir as mybir
    import concourse.tile as tile

    dt = mybir.dt
    AF = mybir.ActivationFunctionType
    ALU = mybir.AluOpType
    f32, bf16 = dt.float32, dt.bfloat16

    nc = bacc.Bacc("TRN2", target_bir_lowering=False, debug=False,
                   num_devices=NCORES)

    def din(name, shape, dty):
        return nc.dram_tensor(name, shape, dty, kind="ExternalInput").ap()

    xin_d = din("xin", [96, 2 * WA], bf16)
    wgk_d = din("wgk", [96, L * 2 * 32], bf16)    # 3-tap lhsT per (l, fn)
    wdstk_d = din("wdstk", [128, L * 128], bf16)  # block-diag dense lhsT
    wskp_d = din("wskp", [128, NGRP * 4 * 128], bf16)  # skip lhsT per (grp, m)
    wp2_d = din("wp2", [128, 8 * 128], bf16)      # Wp2 lhsT per (q, p)
    ones_d = din("ones", [128, 128], bf16)        # sum-broadcast lhsT
    btt_d = din("btt", [128, L], f32)             # bt stacked x4
    bst_d = din("bst", [128, L], f32)             # bs stacked x4
    bdc_d = din("bdc", [64, L], f32)              # bdense (rows 32:64 used)
    hb_d = din("hb", [128, 4], f32)               # h bias per m
    bp2c_d = din("bp2c", [128, 2], f32)           # bp2 per p
    out_d = nc.dram_tensor("out", [QD, V], bf16, kind="ExternalOutput").ap()

    with tile.TileContext(nc) as tc, ExitStack() as top:
        wp = top.enter_context(tc.tile_pool(name="wp", bufs=1))

        def load(d, tag):
            t = wp.tile(list(d.shape), d.dtype, tag=tag, name=tag)
            nc.sync.dma_start(t[:], d[:])
            return t

        wgk = load(wgk_d, "wgk")
        wdstk = load(wdstk_d, "wdstk")
        wskp = load(wskp_d, "wskp")
        wp2 = load(wp2_d, "wp2")
        ones = load(ones_d, "ones")
        btt = load(btt_d, "btt")
        bst = load(bst_d, "bst")
        bdc = load(bdc_d, "bdc")
        hb = load(hb_d, "hb")
        bp2c = load(bp2c_d, "bp2c")

        rings = [wp.tile([128, NB * 512], bf16, tag=f"ring{g}", name=f"ring{g}")
                 for g in range(NGRP)]

        for half in range(2):
            # ---------------- layer phase ----------------
            with ExitStack() as lctx:
                lp = lctx.enter_context(tc.tile_pool(name=f"lp{half}", bufs=1))
                pg = lctx.enter_context(
                    tc.tile_pool(name=f"pg{half}", bufs=1, space="PSUM"))
                pd = lctx.enter_context(
                    tc.tile_pool(name=f"pd{half}", bufs=3, space="PSUM"))
                tu = lctx.enter_context(tc.tile_pool(name=f"tu{half}", bufs=2))

                xx = lp.tile([96, WA], bf16, tag="xx", name="xx")
                nc.sync.dma_start(xx[:], xin_d[:, half * WA:(half + 1) * WA])

                # clear the (rotating, bufs=1) gated psum banks once per half
                # so unwritten rows read as finite values, never NaN
                pgt0 = pg.tile([128, 1024], f32, tag="pgt", name="pgt")
                pgs0 = pg.tile([128, 1024], f32, tag="pgs", name="pgs")
                nc.vector.memset(pgt0[:], 0.0)
                nc.vector.memset(pgs0[:], 0.0)

                pending = []

                def emit_back(l2, ss2, ns2, gm2):
                    # dense conv + x update + tap-replica DMAs for one unit
                    a2 = ss2 % 4
                    lw = wdstk[:, l2 * 128:(l2 + 1) * 128]
                    if a2:
                        blocks = [(0, a2, ns2)]
                    else:
                        blocks = [(b, 0, min(ns2 - 4 * b, 4))
                                  for b in range((ns2 + 3) // 4)]
                    for (b2, ra, nk) in blocks:
                        pdt = pd.tile([128, 512], f32, tag="pd", name="pd")
                        # base partition 0 (PSUM/LDW APs with base>0 are
                        # limited to 32 partitions); rows below 32*ra carry
                        # finite garbage, never evacuated.
                        k1 = 32 * (ra + nk)
                        nc.tensor.matmul(
                            pdt[0:k1, :], lw[0:k1, 0:k1],
                            gm2[0:k1, b2 * 512:(b2 + 1) * 512],
                            start=True, stop=True)
                        for kk in range(nk):
                            r2 = ra + kk
                            F = PAD + 512 * (ss2 + 4 * b2 + kk)
                            if r2 % 2 == 1:
                                nc.vector.scalar_tensor_tensor(
                                    xx[RD:2 * RD, F:F + 512],
                                    pdt[32 * r2:32 * (r2 + 1), :],
                                    bdc[RD:2 * RD, l2:l2 + 1],
                                    xx[RD:2 * RD, F:F + 512],
                                    op0=ALU.add, op1=ALU.add)
                            else:
                                tmp = tu.tile([64, 512], bf16, tag="tmp",
                                              name="tmp")
                                nc.scalar.activation(
                                    tmp[RD:2 * RD, :],
                                    pdt[32 * r2:32 * (r2 + 1), :],
                                    AF.Identity, bias=bdc[RD:2 * RD, l2:l2 + 1])
                                nc.gpsimd.tensor_add(
                                    xx[RD:2 * RD, F:F + 512],
                                    tmp[RD:2 * RD, :],
                                    xx[RD:2 * RD, F:F + 512])
                    dn = DIL[l2 + 1]
                    f0 = PAD + 512 * ss2
                    f1 = PAD + 512 * (ss2 + ns2)
                    nc.sync.dma_start(xx[0:RD, f0 + dn:f1 + dn],
                                      xx[RD:2 * RD, f0:f1])
                    nc.sync.dma_start(xx[2 * RD:3 * RD, f0 - dn:f1 - dn],
                                      xx[RD:2 * RD, f0:f1])

                for l in range(L):
                    G, j = l // 4, l % 4
                    for (ss, ns) in UNITS_L[l]:
                        while len(pending) >= 2:
                            emit_back(*pending.pop(0))
                        a = ss % 4
                        nblk = 1 if a else (ns + 3) // 4
                        # acts always start at partition 0 (PSUM APs with
                        # base>0 are limited to 32 partitions); rows below
                        # 32a hold finite garbage that downstream excludes.
                        p0 = 0
                        p1 = 32 * ((a + ns) if a else min(ns, 4))
                        pgt = pg.tile([128, 1024], f32, tag="pgt", name="pgt")
                        pgs = pg.tile([128, 1024], f32, tag="pgs", name="pgs")
                        for k in range(ns):
                            r = a + k if a else k % 4
                            b = 0 if a else k // 4
                            F = PAD + 512 * (ss + k)
                            for f, pgf in ((0, pgt), (1, pgs)):
                                nc.tensor.matmul(
                                    pgf[32 * r:32 * (r + 1),
                                        512 * b:512 * (b + 1)],
                                    wgk[0:96, (l * 2 + f) * 32:
                                        (l * 2 + f + 1) * 32],
                                    xx[0:96, F:F + 512],
                                    start=True, stop=True,
                                    tile_position=(0, 32 * r))
                        gt = tu.tile([128, 1024], bf16, tag="gt", name="gt")
                        gs = tu.tile([128, 1024], bf16, tag="gs", name="gs")
                        gm = tu.tile([128, 1024], bf16, tag="gm", name="gm")
                        cw = 512 * nblk
                        nc.scalar.activation(gt[p0:p1, 0:cw], pgt[p0:p1, 0:cw],
                                             AF.Tanh, bias=btt[p0:p1, l:l + 1])
                        nc.scalar.activation(gs[p0:p1, 0:cw], pgs[p0:p1, 0:cw],
                                             AF.Sigmoid,
                                             bias=bst[p0:p1, l:l + 1])
                        nc.vector.tensor_mul(gm[p0:p1, 0:cw], gt[p0:p1, 0:cw],
                                             gs[p0:p1, 0:cw])
                        # ring copies (permuted layout, contiguous per row)
                        for r in range(a, a + ns) if a else range(min(ns, 4)):
                            sigs = [ss + k for k in range(ns)
                                    if (a + k if a else k % 4) == r
                                    and SV0 <= ss + k <= SV1]
                            if not sigs:
                                continue
                            b0 = (0 if a else (sigs[0] - ss) // 4)
                            nv = len(sigs)
                            rp = _ring_pos(sigs[0])
                            nc.vector.tensor_copy(
                                rings[G][32 * j:32 * (j + 1),
                                         rp * 512:(rp + nv) * 512],
                                gm[32 * r:32 * (r + 1),
                                   b0 * 512:(b0 + nv) * 512])
                        if l < L - 1:
                            pending.append((l, ss, ns, gm))
                while pending:
                    emit_back(*pending.pop(0))

            # ---------------- skip + post sweep ----------------
            with ExitStack() as pctx:
                sp = pctx.enter_context(tc.tile_pool(name=f"sp{half}", bufs=2))
                rrp = pctx.enter_context(tc.tile_pool(name=f"rr{half}", bufs=6))
                ch = pctx.enter_context(tc.tile_pool(name=f"ch{half}", bufs=1))
                psk = pctx.enter_context(
                    tc.tile_pool(name=f"psk{half}", bufs=5, space="PSUM"))
                pps = pctx.enter_context(
                    tc.tile_pool(name=f"pps{half}", bufs=1, space="PSUM"))
                psb = pctx.enter_context(
                    tc.tile_pool(name=f"psb{half}", bufs=1, space="PSUM"))

                rr_t = {}

                def emit_skip_group(cbg):
                    # 4 cbs; per m accumulate 7 ring matmuls per cb,
                    # cb-innermost for LDWEIGHTS reuse
                    for cb4 in range(4):
                        cb = cbg * 4 + cb4
                        rr_t[cb] = rrp.tile([128, 2048], bf16, tag="rr",
                                            name="rr")
                    for m in range(4):
                        pst = {}
                        for cb4 in range(4):
                            pst[cb4] = psk.tile([128, 512], f32, tag="sk",
                                                name="sk")
                        for g in range(NGRP):
                            nl = min(L - 4 * g, 4)      # layers in group
                            k1 = 32 * nl
                            for cb4 in range(4):
                                cb = cbg * 4 + cb4
                                rp = _ring_pos(SV0 + cb)
                                nc.tensor.matmul(
                                    pst[cb4][:],
                                    wskp[0:k1, (g * 4 + m) * 128:
                                         (g * 4 + m + 1) * 128],
                                    rings[g][0:k1, rp * 512:(rp + 1) * 512],
                                    start=(g == 0), stop=(g == NGRP - 1))
                        for cb4 in range(4):
                            cb = cbg * 4 + cb4
                            nc.vector.tensor_scalar(
                                rr_t[cb][:, m * 512:(m + 1) * 512],
                                pst[cb4][:], hb[:, m:m + 1], 0.0,
                                op0=ALU.add, op1=ALU.max)

                def emit_post1(cbg, oo8, sumball):
                    for cb4 in range(4):
                        cb = cbg * 4 + cb4
                        c8 = cb % 8
                        rr = rr_t.pop(cb)
                        pos = pps.tile([128, 1024], f32, tag="pos", name="pos")
                        for p in range(2):
                            for q in range(4):
                                nc.tensor.matmul(
                                    pos[:, p * 512:(p + 1) * 512],
                                    wp2[:, (q * 2 + p) * 128:
                                        (q * 2 + p + 1) * 128],
                                    rr[:, q * 512:(q + 1) * 512],
                                    start=(q == 0), stop=(q == 3))
                        ee = sp.tile([128, 1024], bf16, tag="ee", name="ee")
                        for p in range(2):
                            nc.scalar.activation(
                                ee[:, p * 512:(p + 1) * 512],
                                pos[:, p * 512:(p + 1) * 512],
                                AF.Exp, bias=bp2c[:, p:p + 1])
                            nc.scalar.activation(
                                oo8[:, c8 * 1024 + p * 512:
                                    c8 * 1024 + (p + 1) * 512],
                                pos[:, p * 512:(p + 1) * 512],
                                AF.Identity, bias=bp2c[:, p:p + 1])
                        psbt = psb.tile([128, 512], f32, tag="sb", name="sb")
                        for p in range(2):
                            nc.tensor.matmul(psbt[:], ones[:],
                                             ee[:, p * 512:(p + 1) * 512],
                                             start=(p == 0), stop=(p == 1))
                        nc.vector.tensor_copy(
                            sumball[:, c8 * 512:(c8 + 1) * 512], psbt[:])

                def emit_ln_post2(chunk, oo8, sumball):
                    lnb = ch.tile([128, 4096], bf16, tag="lnb", name="lnb")
                    nc.scalar.activation(lnb[:], sumball[:], AF.Ln)
                    for c8 in range(8):
                        cb = chunk * 8 + c8
                        oo2 = sp.tile([128, 1024], bf16, tag="oo2", name="oo2")
                        for p in range(2):
                            nc.vector.tensor_sub(
                                oo2[:, p * 512:(p + 1) * 512],
                                oo8[:, c8 * 1024 + p * 512:
                                    c8 * 1024 + (p + 1) * 512],
                                lnb[:, c8 * 512:(c8 + 1) * 512])
                            c0 = half * VH + cb * 512
                            nc.sync.dma_start(
                                out_d[p * 128:(p + 1) * 128, c0:c0 + 512],
                                oo2[:, p * 512:(p + 1) * 512])

                oo8s = [ch.tile([128, 8192], bf16, tag="oo8", name="oo8")
                        for _ in range(2)]
                sbs = [ch.tile([128, 4096], bf16, tag="sba", name="sba")
                       for _ in range(2)]
                for cbg in range(4):
                    emit_skip_group(cbg)
                    if cbg >= 1:
                        emit_post1(cbg - 1, oo8s[(cbg - 1) // 2],
                                   sbs[(cbg - 1) // 2])
                    if cbg == 2:
                        emit_ln_post2(0, oo8s[0], sbs[0])
                emit_post1(3, oo8s[1], sbs[1])
                emit_ln_post2(1, oo8s[1], sbs[1])

    nc.compile()
    return nc


def _prep_host(inputs):
    """Host-side exact fp32 preprocessing: initial conv, weight packing."""
    x = np.asarray(inputs["x"], np.float32)
    Wc = np.asarray(inputs["Wc"], np.float32)
    bc = np.asarray(inputs["bc"], np.float32)
    Wt = np.asarray(inputs["Wt"], np.float32)
    bt = np.asarray(inputs["bt"], np.float32)
    Ws = np.asarray(inputs["Ws"], np.float32)
    bs = np.asarray(inputs["bs"], np.float32)
    Wskip = np.asarray(inputs["Wskip"], np.float32)
    bskip = np.asarray(inputs["bskip"], np.float32)
    Wdense = np.asarray(inputs["Wdense"], np.float32)
    bdense = np.asarray(inputs["bdense"], np.float32)
    Wp1 = np.asarray(inputs["Wp1"], np.float32)
    bp1 = np.asarray(inputs["bp1"], np.float32)
    Wp2 = np.asarray(inputs["Wp2"], np.float32)
    bp2 = np.asarray(inputs["bp2"], np.float32)

    # initial conv (1 -> 32, k=3, pad=1), exact fp32 on host
    x0 = x[0, 0]
    xp = np.pad(x0, (1, 1))
    x1 = (Wc[:, 0, 0:1] * xp[None, 0:T]
          + Wc[:, 0, 1:2] * xp[None, 1:T + 1]
          + Wc[:, 0, 2:3] * xp[None, 2:T + 2]) + bc[:, None]
    xg = np.pad(x1, ((0, 0), (VOFF, VOFF)))

    # layer-0 tap replicas: row strip 0:32 holds x>>d0, 64:96 holds x<<d0
    d0 = DIL[0]
    xin = np.zeros((NCORES, 96, 2 * WA), BF16)
    for c in range(NCORES):
        for hf in range(2):
            s = c * V + hf * VH
            w = xg[:, s:s + WA].astype(BF16)
            o = hf * WA
            xin[c, RD:2 * RD, o:o + WA] = w
            xin[c, 0:RD, o + d0:o + WA] = w[:, :WA - d0]
            xin[c, 2 * RD:3 * RD, o:o + WA - d0] = w[:, d0:]

    wgk = np.zeros((96, L * 2 * 32), np.float32)
    for l in range(L):
        for f, W in ((0, Wt), (1, Ws)):
            for k in range(3):
                wgk[32 * k:32 * (k + 1),
                    (l * 2 + f) * 32:(l * 2 + f + 1) * 32] = W[l, :, :, k].T

    # block-diagonal stacked dense lhsT: rows 32s+j, cols 32s+k = Wdense[l,k,j]
    wdstk = np.zeros((128, L * 128), np.float32)
    for l in range(L):
        for s in range(4):
            wdstk[32 * s:32 * (s + 1),
                  l * 128 + 32 * s:l * 128 + 32 * (s + 1)] = \
                Wdense[l, :, :, 0].T

    W1s = np.einsum("ab,lbc->lac", Wp1[:, :, 0], Wskip[:, :, :, 0])  # [L,512,32]
    wskp = np.zeros((128, NGRP * 4 * 128), np.float32)
    for G in range(NGRP):
        for m in range(4):
            for jj in range(4):
                l = G * 4 + jj
                if l < L:
                    wskp[32 * jj:32 * (jj + 1),
                         (G * 4 + m) * 128:(G * 4 + m + 1) * 128] = \
                        W1s[l, 128 * m:128 * (m + 1), :].T

    wp2 = np.zeros((128, 8 * 128), np.float32)
    for q in range(4):
        for p in range(2):
            wp2[:, (q * 2 + p) * 128:(q * 2 + p + 1) * 128] = \
                Wp2[128 * p:128 * (p + 1), 128 * q:128 * (q + 1), 0].T

    hbias = Wp1[:, :, 0] @ bskip.sum(axis=0) + bp1     # [512]
    hb = hbias.reshape(4, 128).T.copy()                # [128, 4]

    shared = {
        "wgk": wgk.astype(BF16),
        "wdstk": wdstk.astype(BF16),
        "wskp": wskp.astype(BF16),
        "wp2": wp2.astype(BF16),
        "ones": np.ones((128, 128), BF16),
        "btt": np.ascontiguousarray(np.tile(bt.T, (4, 1)).astype(np.float32)),
        "bst": np.ascontiguousarray(np.tile(bs.T, (4, 1)).astype(np.float32)),
        "bdc": np.ascontiguousarray(np.tile(bdense.T, (2, 1)).astype(np.float32)),
        "hb": np.ascontiguousarray(hb.astype(np.float32)),
        "bp2c": np.ascontiguousarray(bp2.reshape(2, 128).T.astype(np.float32)),
    }
    return xin, shared


def kernel(**inputs):
    from concourse.bass_utils import run_bass_kernel_spmd

    xin, shared = _prep_host(inputs)
    if "nc" not in _cache:
        _cache["nc"] = _build()
    nc = _cache["nc"]

    in_maps = [dict(shared, xin=np.ascontiguousarray(xin[c]))
               for c in range(NCORES)]
    res = run_bass_kernel_spmd(nc, in_maps, core_ids=list(range(NCORES)))

    _last_run["nc"] = nc
    _last_run["in_maps"] = in_maps

    out = np.empty((1, QD, T), np.float32)
    for c in range(NCORES):
        out[0, :, c * V:(c + 1) * V] = res.results[c]["out"].astype(np.float32)
    return out


# revision 14
# speedup vs baseline: 1.1256x; 1.1256x over previous
"""Trainium2 Bass kernel for a WaveNet-style dilated-conv stack (v4).

Network (per reference):
  x1 = conv1d(x, Wc, bc, d=1, pad=1)                      # 1 -> 32, host-side
  for l in 27 layers, d = 2^(l%9):
      g = tanh(conv(x, Wt_l, d)) * sigmoid(conv(x, Ws_l, d))   # 32->32, k=3, pad=d
      skip += conv1x1(g, Wskip_l)                              # 32->512
      x = conv1x1(g, Wdense_l) + x
  out = conv1x1(relu(conv1x1(skip, Wp1)), Wp2)            # 512->512->256
  return log_softmax(out, axis=channels)

v4 design (8 cores, sequence-parallel, no cross-core comm):
  - Wp1 folded into skip weights (W1s = Wp1 @ Wskip) on host.
  - Per core 2 halves of 8192 cols; per-layer SHRINKING windows: layer l
    computes radius R_l = 512*ceil(S_{l+1}/512) (S = suffix dilation sum),
    R = 1536/1024/512/0.  Strip grid anchored at PAD; units are 4-strip
    aligned (8-strip super units + small boundary units).
  - Gated convs: one K=96 matmul per (strip, fn), col-tiled 4-way by
    sigma%4 into [128, 1024] f32 psum per fn; tanh/sigmoid at N=1024.
  - g stored stacked [128, 1024] bf16; ring copies to 7 per-group ring
    tiles [128, 8192] in a PERMUTED valid-strip order so each copy is one
    contiguous [32, <=1024] 4x-mode DVE op.  Rings for all 27 layers live
    simultaneously -> NO h accumulator, single skip sweep at end of half.
  - Dense conv: ONE stacked matmul (block-diag lhsT, M=128, K=32*strips)
    per 2048-block; evac+residual per strip: rows 1,3 -> DVE stt (psum
    cross-partition ok), rows 0,2 -> ScalarE Identity+bias to a [64,512]
    tmp at rows 32:64 + GpSimd tensor_add into xx (all same-partition).
  - Tap replicas (x>>d, x<<d on partition strips 0:32 / 64:96) maintained
    by 2 SBUF->SBUF DMAs per unit on the Sync queue.
  - Skip+post sweep per half: per 4-cb group, per m: 7 K-chunk matmuls
    (K=96 for the last 3-layer group: avoids reading unwritten ring rows),
    cb-inner for LDWEIGHTS reuse; DVE relu(x+hb) evac to rr.  Post per cb:
    8 Wp2 matmuls -> pos [128,1024] psum, Exp/Identity acts (bias bp2),
    sum-broadcast via 2 accumulating ones-matmuls, chunked Ln over 8 cbs
    (exp/ln stay in one table set per chunk -> no ACT_TABLE thrash), final
    log_softmax subtract on DVE, bf16 output DMA (host casts to f32).
"""

import numpy as np
import ml_dtypes

BF16 = ml_dtypes.bfloat16

DIL = [2 ** i for i in range(9)] * 3
L = len(DIL)            # 27
RD, SD, QD = 32, 512, 256
T = 131072
NCORES = 8
V = T // NCORES         # 16384 per core
VH = V // 2             # 8192 per half
HALO = 1536
PAD = 256
WH = VH + 2 * HALO      # 11264 max computed window per half
WA = WH + 2 * PAD       # 11776 allocated width per half
VOFF = HALO + PAD       # 1792 valid-region offset
NB = VH // 512          # 16 valid 512-col strips per half
NGRP = (L + 3) // 4     # 7 groups of (up to) 4 layers

# per-layer computed radius (cols), 512-aligned; S_{l+1} = sum(DIL[l+1:])
_S = [sum(DIL[i + 1:]) for i in range(L)]
R_L = [512 * ((s + 511) // 512) for s in _S]      # 1536x8, 1024x9, 512x9, 0
for _l in range(L):
    assert R_L[_l] >= _S[_l] and R_L[_l] <= HALO

# valid strips: global strip sigma covers flat cols [PAD+512s, PAD+512s+512)
SV0 = (VOFF - PAD) // 512        # 3 = first valid strip
SV1 = SV0 + NB - 1               # 18 = last valid strip


def _units_for(R):
    """Unit list [(sigma_start, n_strips)] for radius R; interior units are
    4-strip aligned, boundary-left unit stays within one 4-strip block."""
    s0 = SV0 - R // 512
    s1 = SV1 + 1 + R // 512
    units = []
    sa = s0
    if s0 % 4:
        ns = min(4 - s0 % 4, s1 - s0)
        units.append((s0, ns))
        sa = s0 + ns
    while sa < s1:
        ns = min(4, s1 - sa)
        units.append((sa, ns))
        sa += ns
    return units


def _pair_units(units):
    """Group consecutive 4-aligned units into pairs sharing one gm tile."""
    groups, i = [], 0
    while i < len(units):
        ss, ns = units[i]
        if (ss % 4 == 0 and i + 1 < len(units)
                and units[i + 1][0] % 4 == 0):
            groups.append([(ss, ns, 0), (units[i + 1][0], units[i + 1][1], 1)])
            i += 2
        else:
            groups.append([(ss, ns, 0)])
            i += 1
    return groups


UNITS_L = [_units_for(R) for R in R_L]
PAIRS_L = [_pair_units(u) for u in UNITS_L]


def _ring_pos(sigma):
    """Permuted ring column position for valid strip sigma (see docstring)."""
    r = sigma % 4
    b = sigma // 4
    bmin = 1 if r < 3 else 0
    return r * 4 + (b - bmin)


_cache = {}
_last_run = {}


def _build():
    from contextlib import ExitStack

    import concourse.bacc as bacc
    import concourse.mybir as mybir
    import concourse.tile as tile

    dt = mybir.dt
    AF = mybir.ActivationFunctionType
    ALU = mybir.AluOpType
    f32, bf16 = dt.float32, dt.bfloat16

    nc = bacc.Bacc("TRN2", target_bir_lowering=False, debug=False,
                   num_devices=NCORES)

    def din(name, shape, dty):
        return nc.dram_tensor(name, shape, dty, kind="ExternalInput").ap()

    xin_d = din("xin", [96, 2 * WA], bf16)
    wgk_d = din("wgk", [96, L * 2 * 32], bf16)    # 3-tap lhsT per (l, fn)
    wdstk_d = din("wdstk", [128, L * 128], bf16)  # block-diag dense lhsT
    wskp_d = din("wskp", [128, NGRP * 4 * 128], bf16)  # skip lhsT per (grp, m)
    wp2_d = din("wp2", [128, 8 * 128], bf16)      # Wp2 lhsT per (q, p)
    ones_d = din("ones", [128, 128], bf16)        # sum-broadcast lhsT
    btt_d = din("btt", [128, L], f32)             # bt stacked x4
    bst_d = din("bst", [128, L], f32)             # bs stacked x4
    bdc_d = din("bdc", [64, L], f32)              # bdense (rows 32:64 used)
    hb_d = din("hb", [128, 4], f32)               # h bias per m
    bp2c_d = din("bp2c", [128, 2], f32)           # bp2 per p
    out_d = nc.dram_tensor("out", [QD, V], bf16, kind="ExternalOutput").ap()

    with tile.TileContext(nc) as tc, ExitStack() as top:
        wp = top.enter_context(tc.tile_pool(name="wp", bufs=1))

        def load(d, tag):
            t = wp.tile(list(d.shape), d.dtype, tag=tag, name=tag)
            nc.sync.dma_start(t[:], d[:])
            return t

        wgk = load(wgk_d, "wgk")
        wdstk = load(wdstk_d, "wdstk")
        wskp = load(wskp_d, "wskp")
        wp2 = load(wp2_d, "wp2")
        ones = load(ones_d, "ones")
        btt = load(btt_d, "btt")
        bst = load(bst_d, "bst")
        bdc = load(bdc_d, "bdc")
        hb = load(hb_d, "hb")
        bp2c = load(bp2c_d, "bp2c")

        rings = [wp.tile([128, NB * 512], bf16, tag=f"ring{g}", name=f"ring{g}")
                 for g in range(NGRP)]

        for half in range(2):
            # ---------------- layer phase ----------------
            with ExitStack() as lctx:
                lp = lctx.enter_context(tc.tile_pool(name=f"lp{half}", bufs=1))
                pg = lctx.enter_context(
                    tc.tile_pool(name=f"pg{half}", bufs=2, space="PSUM"))
                pd = lctx.enter_context(
                    tc.tile_pool(name=f"pd{half}", bufs=3, space="PSUM"))
                tu = lctx.enter_context(tc.tile_pool(name=f"tu{half}", bufs=2))

                xx = lp.tile([96, WA], bf16, tag="xx", name="xx")
                nc.sync.dma_start(xx[:], xin_d[:, half * WA:(half + 1) * WA])

                # clear the rotating gated psum banks once per half so
                # unwritten rows read as finite values, never NaN
                for _ in range(2):
                    pgt0 = pg.tile([128, 512], f32, tag="pgt", name="pgt")
                    pgs0 = pg.tile([128, 512], f32, tag="pgs", name="pgs")
                    nc.vector.memset(pgt0[:], 0.0)
                    nc.vector.memset(pgs0[:], 0.0)

                pending = []
                uidx = [0]

                def emit_back(l2, ss2, ns2, gm2, off2):
                    # dense conv + x update + tap-replica DMAs for one unit
                    a2 = ss2 % 4
                    lw = wdstk[:, l2 * 128:(l2 + 1) * 128]
                    k1 = 32 * (a2 + ns2)
                    pdt = pd.tile([128, 512], f32, tag="pd", name="pd")
                    nc.tensor.matmul(
                        pdt[0:k1, :], lw[0:k1, 0:k1],
                        gm2[0:k1, off2 * 512:(off2 + 1) * 512],
                        start=True, stop=True)
                    for kk in range(ns2):
                        r2 = a2 + kk
                        F = PAD + 512 * (ss2 + kk)
                        if r2 % 2 == 1:
                            nc.vector.scalar_tensor_tensor(
                                xx[RD:2 * RD, F:F + 512],
                                pdt[32 * r2:32 * (r2 + 1), :],
                                bdc[RD:2 * RD, l2:l2 + 1],
                                xx[RD:2 * RD, F:F + 512],
                                op0=ALU.add, op1=ALU.add)
                        else:
                            tmp = tu.tile([64, 512], bf16, tag="tmp",
                                          name="tmp")
                            nc.scalar.activation(
                                tmp[RD:2 * RD, :],
                                pdt[32 * r2:32 * (r2 + 1), :],
                                AF.Identity, bias=bdc[RD:2 * RD, l2:l2 + 1])
                            nc.gpsimd.tensor_add(
                                xx[RD:2 * RD, F:F + 512],
                                tmp[RD:2 * RD, :],
                                xx[RD:2 * RD, F:F + 512])
                    dn = DIL[l2 + 1]
                    f0 = PAD + 512 * ss2
                    f1 = PAD + 512 * (ss2 + ns2)
                    uidx[0] += 1
                    qa, qb = ((nc.sync, nc.gpsimd) if uidx[0] % 2
                              else (nc.gpsimd, nc.sync))
                    qa.dma_start(xx[0:RD, f0 + dn:f1 + dn],
                                 xx[RD:2 * RD, f0:f1])
                    qb.dma_start(xx[2 * RD:3 * RD, f0 - dn:f1 - dn],
                                 xx[RD:2 * RD, f0:f1])

                for l in range(L):
                    G, j = l // 4, l % 4
                    for grp in PAIRS_L[l]:
                        while len(pending) >= 3:
                            emit_back(*pending.pop(0))
                        # gated matmuls for all units of the pair first
                        pgfs = []
                        for (ss, ns, off) in grp:
                            a = ss % 4
                            pgt = pg.tile([128, 512], f32, tag="pgt",
                                          name="pgt")
                            pgs = pg.tile([128, 512], f32, tag="pgs",
                                          name="pgs")
                            pgfs.append((pgt, pgs))
                            for k in range(ns):
                                r = a + k
                                F = PAD + 512 * (ss + k)
                                for f, pgf in ((0, pgt), (1, pgs)):
                                    nc.tensor.matmul(
                                        pgf[32 * r:32 * (r + 1), :],
                                        wgk[0:96, (l * 2 + f) * 32:
                                            (l * 2 + f + 1) * 32],
                                        xx[0:96, F:F + 512],
                                        start=True, stop=True,
                                        tile_position=(0, 32 * r))
                        gt = tu.tile([128, 1024], bf16, tag="gt", name="gt")
                        gs = tu.tile([128, 1024], bf16, tag="gs", name="gs")
                        gm = tu.tile([128, 1024], bf16, tag="gm", name="gm")
                        p1m = 0
                        for (ss, ns, off), (pgt, pgs) in zip(grp, pgfs):
                            a = ss % 4
                            p1 = 32 * (a + ns)
                            p1m = max(p1m, p1)
                            nc.scalar.activation(
                                gt[0:p1, off * 512:(off + 1) * 512],
                                pgt[0:p1, :], AF.Tanh,
                                bias=btt[0:p1, l:l + 1])
                            nc.scalar.activation(
                                gs[0:p1, off * 512:(off + 1) * 512],
                                pgs[0:p1, :], AF.Sigmoid,
                                bias=bst[0:p1, l:l + 1])
                        cw = 512 * len(grp)
                        nc.vector.tensor_mul(gm[0:p1m, 0:cw], gt[0:p1m, 0:cw],
                                             gs[0:p1m, 0:cw])
                        # ring copies (permuted layout -> contiguous per row)
                        for r in range(4):
                            vs = []   # (off, sigma) valid strips in row r
                            for (ss, ns, off) in grp:
                                a = ss % 4
                                for k in range(ns):
                                    sig = ss + k
                                    if a + k == r and SV0 <= sig <= SV1:
                                        vs.append((off, sig))
                            if not vs:
                                continue
                            rp = _ring_pos(vs[0][1])
                            nv = len(vs)
                            nc.vector.tensor_copy(
                                rings[G][32 * j:32 * (j + 1),
                                         rp * 512:(rp + nv) * 512],
                                gm[32 * r:32 * (r + 1),
                                   vs[0][0] * 512:(vs[0][0] + nv) * 512])
                        if l < L - 1:
                            for (ss, ns, off) in grp:
                                pending.append((l, ss, ns, gm, off))
                while pending:
                    emit_back(*pending.pop(0))

            # ---------------- skip + post sweep ----------------
            with ExitStack() as pctx:
                sp = pctx.enter_context(tc.tile_pool(name=f"sp{half}", bufs=2))
                rrp = pctx.enter_context(tc.tile_pool(name=f"rr{half}", bufs=6))
                ch = pctx.enter_context(tc.tile_pool(name=f"ch{half}", bufs=1))
                psk = pctx.enter_context(
                    tc.tile_pool(name=f"psk{half}", bufs=5, space="PSUM"))
                pps = pctx.enter_context(
                    tc.tile_pool(name=f"pps{half}", bufs=1, space="PSUM"))
                psb = pctx.enter_context(
                    tc.tile_pool(name=f"psb{half}", bufs=1, space="PSUM"))

                rr_t = {}

                def emit_skip_group(cbg):
                    # 4 cbs; per m accumulate 7 ring matmuls per cb,
                    # cb-innermost for LDWEIGHTS reuse
                    for cb4 in range(4):
                        cb = cbg * 4 + cb4
                        rr_t[cb] = rrp.tile([128, 2048], bf16, tag="rr",
                                            name="rr")
                    for m in range(4):
                        pst = {}
                        for cb4 in range(4):
                            pst[cb4] = psk.tile([128, 512], f32, tag="sk",
                                                name="sk")
                        for g in range(NGRP):
                            nl = min(L - 4 * g, 4)      # layers in group
                            k1 = 32 * nl
                            for cb4 in range(4):
                                cb = cbg * 4 + cb4
                                rp = _ring_pos(SV0 + cb)
                                nc.tensor.matmul(
                                    pst[cb4][:],
                                    wskp[0:k1, (g * 4 + m) * 128:
                                         (g * 4 + m + 1) * 128],
                                    rings[g][0:k1, rp * 512:(rp + 1) * 512],
                                    start=(g == 0), stop=(g == NGRP - 1))
                        for cb4 in range(4):
                            cb = cbg * 4 + cb4
                            nc.vector.tensor_scalar(
                                rr_t[cb][:, m * 512:(m + 1) * 512],
                                pst[cb4][:], hb[:, m:m + 1], 0.0,
                                op0=ALU.add, op1=ALU.max)

                def emit_post1(cbg, oo8, sumball):
                    for cb4 in range(4):
                        cb = cbg * 4 + cb4
                        c8 = cb % 8
                        rr = rr_t.pop(cb)
                        pos = pps.tile([128, 1024], f32, tag="pos", name="pos")
                        for p in range(2):
                            for q in range(4):
                                nc.tensor.matmul(
                                    pos[:, p * 512:(p + 1) * 512],
                                    wp2[:, (q * 2 + p) * 128:
                                        (q * 2 + p + 1) * 128],
                                    rr[:, q * 512:(q + 1) * 512],
                                    start=(q == 0), stop=(q == 3))
                        ee = sp.tile([128, 1024], bf16, tag="ee", name="ee")
                        for p in range(2):
                            nc.scalar.activation(
                                ee[:, p * 512:(p + 1) * 512],
                                pos[:, p * 512:(p + 1) * 512],
                                AF.Exp, bias=bp2c[:, p:p + 1])
                            nc.scalar.activation(
                                oo8[:, c8 * 1024 + p * 512:
                                    c8 * 1024 + (p + 1) * 512],
                                pos[:, p * 512:(p + 1) * 512],
                                AF.Identity, bias=bp2c[:, p:p + 1])
                        psbt = psb.tile([128, 512], f32, tag="sb", name="sb")
                        for p in range(2):
                            nc.tensor.matmul(psbt[:], ones[:],
                                             ee[:, p * 512:(p + 1) * 512],
                                             start=(p == 0), stop=(p == 1))
                        nc.vector.tensor_copy(
                            sumball[:, c8 * 512:(c8 + 1) * 512], psbt[:])

                def emit_ln_post2(chunk, oo8, sumball):
                    lnb = ch.tile([128, 4096], bf16, tag="lnb", name="lnb")
                    nc.scalar.activation(lnb[:], sumball[:], AF.Ln)
                    for c8 in range(8):
                        cb = chunk * 8 + c8
                        oo2 = sp.tile([128, 1024], bf16, tag="oo2", name="oo2")
                        for p in range(2):
                            nc.vector.tensor_sub(
                                oo2[:, p * 512:(p + 1) * 512],
                                oo8[:, c8 * 1024 + p * 512:
                                    c8 * 1024 + (p + 1) * 512],
                                lnb[:, c8 * 512:(c8 + 1) * 512])
                            c0 = half * VH + cb * 512
                            nc.sync.dma_start(
                                out_d[p * 128:(p + 1) * 128, c0:c0 + 512],
                                oo2[:, p * 512:(p + 1) * 512])

                oo8s = [ch.tile([128, 8192], bf16, tag="oo8", name="oo8")
                        for _ in range(2)]
                sbs = [ch.tile([128, 4096], bf16, tag="sba", name="sba")
                       for _ in range(2)]
                for cbg in range(4):
                    emit_skip_group(cbg)
                    if cbg >= 1:
                        emit_post1(cbg - 1, oo8s[(cbg - 1) // 2],
                                   sbs[(cbg - 1) // 2])
                    if cbg == 2:
                        emit_ln_post2(0, oo8s[0], sbs[0])
                emit_post1(3, oo8s[1], sbs[1])
                emit_ln_post2(1, oo8s[1], sbs[1])

    nc.compile()
    return nc


def _prep_host(inputs):
    """Host-side exact fp32 preprocessing: initial conv, weight packing."""
    x = np.asarray(inputs["x"], np.float32)
    Wc = np.asarray(inputs["Wc"], np.float32)
    bc = np.asarray(inputs["bc"], np.float32)
    Wt = np.asarray(inputs["Wt"], np.float32)
    bt = np.asarray(inputs["bt"], np.float32)
    Ws = np.asarray(inputs["Ws"], np.float32)
    bs = np.asarray(inputs["bs"], np.float32)
    Wskip = np.asarray(inputs["Wskip"], np.float32)
    bskip = np.asarray(inputs["bskip"], np.float32)
    Wdense = np.asarray(inputs["Wdense"], np.float32)
    bdense = np.asarray(inputs["bdense"], np.float32)
    Wp1 = np.asarray(inputs["Wp1"], np.float32)
    bp1 = np.asarray(inputs["bp1"], np.float32)
    Wp2 = np.asarray(inputs["Wp2"], np.float32)
    bp2 = np.asarray(inputs["bp2"], np.float32)

    # initial conv (1 -> 32, k=3, pad=1), exact fp32 on host
    x0 = x[0, 0]
    xp = np.pad(x0, (1, 1))
    x1 = (Wc[:, 0, 0:1] * xp[None, 0:T]
          + Wc[:, 0, 1:2] * xp[None, 1:T + 1]
          + Wc[:, 0, 2:3] * xp[None, 2:T + 2]) + bc[:, None]
    xg = np.pad(x1, ((0, 0), (VOFF, VOFF)))

    # layer-0 tap replicas: row strip 0:32 holds x>>d0, 64:96 holds x<<d0
    d0 = DIL[0]
    xin = np.zeros((NCORES, 96, 2 * WA), BF16)
    for c in range(NCORES):
        for hf in range(2):
            s = c * V + hf * VH
            w = xg[:, s:s + WA].astype(BF16)
            o = hf * WA
            xin[c, RD:2 * RD, o:o + WA] = w
            xin[c, 0:RD, o + d0:o + WA] = w[:, :WA - d0]
            xin[c, 2 * RD:3 * RD, o:o + WA - d0] = w[:, d0:]

    wgk = np.zeros((96, L * 2 * 32), np.float32)
    for l in range(L):
        for f, W in ((0, Wt), (1, Ws)):
            for k in range(3):
                wgk[32 * k:32 * (k + 1),
                    (l * 2 + f) * 32:(l * 2 + f + 1) * 32] = W[l, :, :, k].T

    # block-diagonal stacked dense lhsT: rows 32s+j, cols 32s+k = Wdense[l,k,j]
    wdstk = np.zeros((128, L * 128), np.float32)
    for l in range(L):
        for s in range(4):
            wdstk[32 * s:32 * (s + 1),
                  l * 128 + 32 * s:l * 128 + 32 * (s + 1)] = \
                Wdense[l, :, :, 0].T

    W1s = np.einsum("ab,lbc->lac", Wp1[:, :, 0], Wskip[:, :, :, 0])  # [L,512,32]
    wskp = np.zeros((128, NGRP * 4 * 128), np.float32)
    for G in range(NGRP):
        for m in range(4):
            for jj in range(4):
                l = G * 4 + jj
                if l < L:
                    wskp[32 * jj:32 * (jj + 1),
                         (G * 4 + m) * 128:(G * 4 + m + 1) * 128] = \
                        W1s[l, 128 * m:128 * (m + 1), :].T

    wp2 = np.zeros((128, 8 * 128), np.float32)
    for q in range(4):
        for p in range(2):
            wp2[:, (q * 2 + p) * 128:(q * 2 + p + 1) * 128] = \
                Wp2[128 * p:128 * (p + 1), 128 * q:128 * (q + 1), 0].T

    hbias = Wp1[:, :, 0] @ bskip.sum(axis=0) + bp1     # [512]
    hb = hbias.reshape(4, 128).T.copy()                # [128, 4]

    shared = {
        "wgk": wgk.astype(BF16),
        "wdstk": wdstk.astype(BF16),
        "wskp": wskp.astype(BF16),
        "wp2": wp2.astype(BF16),
        "ones": np.ones((128, 128), BF16),
        "btt": np.ascontiguousarray(np.tile(bt.T, (4, 1)).astype(np.float32)),
        "bst": np.ascontiguousarray(np.tile(bs.T, (4, 1)).astype(np.float32)),
        "bdc": np.ascontiguousarray(np.tile(bdense.T, (2, 1)).astype(np.float32)),
        "hb": np.ascontiguousarray(hb.astype(np.float32)),
        "bp2c": np.ascontiguousarray(bp2.reshape(2, 128).T.astype(np.float32)),
    }
    return xin, shared


def kernel(**inputs):
    from concourse.bass_utils import run_bass_kernel_spmd

    xin, shared = _prep_host(inputs)
    if "nc" not in _cache:
        _cache["nc"] = _build()
    nc = _cache["nc"]

    in_maps = [dict(shared, xin=np.ascontiguousarray(xin[c]))
               for c in range(NCORES)]
    res = run_bass_kernel_spmd(nc, in_maps, core_ids=list(range(NCORES)))

    _last_run["nc"] = nc
    _last_run["in_maps"] = in_maps

    out = np.empty((1, QD, T), np.float32)
    for c in range(NCORES):
        out[0, :, c * V:(c + 1) * V] = res.results[c]["out"].astype(np.float32)
    return out


# revision 16
# speedup vs baseline: 1.2268x; 1.0898x over previous
"""Trainium2 Bass kernel for a WaveNet-style dilated-conv stack (v4).

Network (per reference):
  x1 = conv1d(x, Wc, bc, d=1, pad=1)                      # 1 -> 32, host-side
  for l in 27 layers, d = 2^(l%9):
      g = tanh(conv(x, Wt_l, d)) * sigmoid(conv(x, Ws_l, d))   # 32->32, k=3, pad=d
      skip += conv1x1(g, Wskip_l)                              # 32->512
      x = conv1x1(g, Wdense_l) + x
  out = conv1x1(relu(conv1x1(skip, Wp1)), Wp2)            # 512->512->256
  return log_softmax(out, axis=channels)

v4 design (8 cores, sequence-parallel, no cross-core comm):
  - Wp1 folded into skip weights (W1s = Wp1 @ Wskip) on host.
  - Per core 2 halves of 8192 cols; per-layer SHRINKING windows: layer l
    computes radius R_l = 512*ceil(S_{l+1}/512) (S = suffix dilation sum),
    R = 1536/1024/512/0.  Strip grid anchored at PAD; units are 4-strip
    aligned (8-strip super units + small boundary units).
  - Gated convs: one K=96 matmul per (strip, fn), col-tiled 4-way by
    sigma%4 into [128, 1024] f32 psum per fn; tanh/sigmoid at N=1024.
  - g stored stacked [128, 1024] bf16; ring copies to 7 per-group ring
    tiles [128, 8192] in a PERMUTED valid-strip order so each copy is one
    contiguous [32, <=1024] 4x-mode DVE op.  Rings for all 27 layers live
    simultaneously -> NO h accumulator, single skip sweep at end of half.
  - Dense conv: ONE stacked matmul (block-diag lhsT, M=128, K=32*strips)
    per 2048-block; evac+residual per strip: rows 1,3 -> DVE stt (psum
    cross-partition ok), rows 0,2 -> ScalarE Identity+bias to a [64,512]
    tmp at rows 32:64 + GpSimd tensor_add into xx (all same-partition).
  - Tap replicas (x>>d, x<<d on partition strips 0:32 / 64:96) maintained
    by 2 SBUF->SBUF DMAs per unit on the Sync queue.
  - Skip+post sweep per half: per 4-cb group, per m: 7 K-chunk matmuls
    (K=96 for the last 3-layer group: avoids reading unwritten ring rows),
    cb-inner for LDWEIGHTS reuse; DVE relu(x+hb) evac to rr.  Post per cb:
    8 Wp2 matmuls -> pos [128,1024] psum, Exp/Identity acts (bias bp2),
    sum-broadcast via 2 accumulating ones-matmuls, chunked Ln over 8 cbs
    (exp/ln stay in one table set per chunk -> no ACT_TABLE thrash), final
    log_softmax subtract on DVE, bf16 output DMA (host casts to f32).
"""

import numpy as np
import ml_dtypes

BF16 = ml_dtypes.bfloat16

DIL = [2 ** i for i in range(9)] * 3
L = len(DIL)            # 27
RD, SD, QD = 32, 512, 256
T = 131072
NCORES = 8
V = T // NCORES         # 16384 per core
VH = V // 2             # 8192 per half
HALO = 1536
PAD = 256
WH = VH + 2 * HALO      # 11264 max computed window per half
WA = WH + 2 * PAD       # 11776 allocated width per half
VOFF = HALO + PAD       # 1792 valid-region offset
NB = VH // 512          # 16 valid 512-col strips per half
NGRP = (L + 3) // 4     # 7 groups of (up to) 4 layers

# per-layer computed radius (cols), 512-aligned; S_{l+1} = sum(DIL[l+1:])
_S = [sum(DIL[i + 1:]) for i in range(L)]
R_L = [512 * ((s + 511) // 512) for s in _S]      # 1536x8, 1024x9, 512x9, 0
for _l in range(L):
    assert R_L[_l] >= _S[_l] and R_L[_l] <= HALO

# valid strips: global strip sigma covers flat cols [PAD+512s, PAD+512s+512)
SV0 = (VOFF - PAD) // 512        # 3 = first valid strip
SV1 = SV0 + NB - 1               # 18 = last valid strip


def _units_for(R):
    """Unit list [(sigma_start, n_strips)] for radius R; interior units are
    4-strip aligned, boundary-left unit stays within one 4-strip block."""
    s0 = SV0 - R // 512
    s1 = SV1 + 1 + R // 512
    units = []
    sa = s0
    if s0 % 4:
        ns = min(4 - s0 % 4, s1 - s0)
        units.append((s0, ns))
        sa = s0 + ns
    while sa < s1:
        ns = min(4, s1 - sa)
        units.append((sa, ns))
        sa += ns
    return units


def _pair_units(units):
    """Group consecutive 4-aligned units into pairs sharing one gm tile."""
    groups, i = [], 0
    while i < len(units):
        ss, ns = units[i]
        if (ss % 4 == 0 and i + 1 < len(units)
                and units[i + 1][0] % 4 == 0):
            groups.append([(ss, ns, 0), (units[i + 1][0], units[i + 1][1], 1)])
            i += 2
        else:
            groups.append([(ss, ns, 0)])
            i += 1
    return groups


UNITS_L = [_units_for(R) for R in R_L]
PAIRS_L = [_pair_units(u) for u in UNITS_L]


def _ring_pos(sigma):
    """Permuted ring column position for valid strip sigma (see docstring)."""
    r = sigma % 4
    b = sigma // 4
    bmin = 1 if r < 3 else 0
    return r * 4 + (b - bmin)


_cache = {}
_last_run = {}


def _build():
    from contextlib import ExitStack

    import concourse.bacc as bacc
    import concourse.mybir as mybir
    import concourse.tile as tile

    dt = mybir.dt
    AF = mybir.ActivationFunctionType
    ALU = mybir.AluOpType
    f32, bf16 = dt.float32, dt.bfloat16

    nc = bacc.Bacc("TRN2", target_bir_lowering=False, debug=False,
                   num_devices=NCORES)

    def din(name, shape, dty):
        return nc.dram_tensor(name, shape, dty, kind="ExternalInput").ap()

    xin_d = din("xin", [96, 2 * WA], bf16)
    wgk_d = din("wgk", [96, L * 2 * 32], bf16)    # 3-tap lhsT per (l, fn)
    wdstk_d = din("wdstk", [128, L * 128], bf16)  # block-diag dense lhsT
    wskp_d = din("wskp", [128, NGRP * 4 * 128], bf16)  # skip lhsT per (grp, m)
    wp2_d = din("wp2", [128, 8 * 128], bf16)      # Wp2 lhsT per (q, p)
    ones_d = din("ones", [128, 128], bf16)        # sum-broadcast lhsT
    btt_d = din("btt", [128, L], f32)             # bt stacked x4
    bst_d = din("bst", [128, L], f32)             # bs stacked x4
    bdc_d = din("bdc", [64, L], f32)              # bdense (rows 32:64 used)
    hb_d = din("hb", [128, 4], f32)               # h bias per m
    bp2c_d = din("bp2c", [128, 2], f32)           # bp2 per p
    out_d = nc.dram_tensor("out", [QD, V], bf16, kind="ExternalOutput").ap()

    with tile.TileContext(nc) as tc, ExitStack() as top:
        wp = top.enter_context(tc.tile_pool(name="wp", bufs=1))

        def load(d, tag):
            t = wp.tile(list(d.shape), d.dtype, tag=tag, name=tag)
            nc.sync.dma_start(t[:], d[:])
            return t

        wgk = load(wgk_d, "wgk")
        wdstk = load(wdstk_d, "wdstk")
        wskp = load(wskp_d, "wskp")
        wp2 = load(wp2_d, "wp2")
        ones = load(ones_d, "ones")
        btt = load(btt_d, "btt")
        bst = load(bst_d, "bst")
        bdc = load(bdc_d, "bdc")
        hb = load(hb_d, "hb")
        bp2c = load(bp2c_d, "bp2c")

        rings = [wp.tile([128, NB * 512], bf16, tag=f"ring{g}", name=f"ring{g}")
                 for g in range(NGRP)]

        for half in range(2):
            # ---------------- layer phase ----------------
            with ExitStack() as lctx:
                lp = lctx.enter_context(tc.tile_pool(name=f"lp{half}", bufs=1))
                pg = lctx.enter_context(
                    tc.tile_pool(name=f"pg{half}", bufs=2, space="PSUM"))
                pd = lctx.enter_context(
                    tc.tile_pool(name=f"pd{half}", bufs=3, space="PSUM"))
                tu = lctx.enter_context(tc.tile_pool(name=f"tu{half}", bufs=2))

                xx = lp.tile([96, WA], bf16, tag="xx", name="xx")
                nc.sync.dma_start(xx[:], xin_d[:, half * WA:(half + 1) * WA])

                # clear the rotating gated psum banks once per half so
                # unwritten rows read as finite values, never NaN
                for _ in range(2):
                    pgt0 = pg.tile([128, 512], f32, tag="pgt", name="pgt")
                    pgs0 = pg.tile([128, 512], f32, tag="pgs", name="pgs")
                    nc.vector.memset(pgt0[:], 0.0)
                    nc.vector.memset(pgs0[:], 0.0)

                pending = []
                uidx = [0]

                def emit_back(l2, ss2, ns2, gm2, off2):
                    # dense conv + x update + tap-replica DMAs for one unit
                    a2 = ss2 % 4
                    lw = wdstk[:, l2 * 128:(l2 + 1) * 128]
                    k1 = 32 * (a2 + ns2)
                    pdt = pd.tile([128, 512], f32, tag="pd", name="pd")
                    nc.tensor.matmul(
                        pdt[0:k1, :], lw[0:k1, 0:k1],
                        gm2[0:k1, off2 * 512:(off2 + 1) * 512],
                        start=True, stop=True)
                    for kk in range(ns2):
                        r2 = a2 + kk
                        F = PAD + 512 * (ss2 + kk)
                        if r2 % 2 == 1:
                            nc.vector.scalar_tensor_tensor(
                                xx[RD:2 * RD, F:F + 512],
                                pdt[32 * r2:32 * (r2 + 1), :],
                                bdc[RD:2 * RD, l2:l2 + 1],
                                xx[RD:2 * RD, F:F + 512],
                                op0=ALU.add, op1=ALU.add)
                        else:
                            tmp = tu.tile([64, 512], bf16, tag="tmp",
                                          name="tmp")
                            nc.scalar.activation(
                                tmp[RD:2 * RD, :],
                                pdt[32 * r2:32 * (r2 + 1), :],
                                AF.Identity, bias=bdc[RD:2 * RD, l2:l2 + 1])
                            nc.gpsimd.tensor_add(
                                xx[RD:2 * RD, F:F + 512],
                                tmp[RD:2 * RD, :],
                                xx[RD:2 * RD, F:F + 512])
                    dn = DIL[l2 + 1]
                    f0 = PAD + 512 * ss2
                    f1 = PAD + 512 * (ss2 + ns2)
                    uidx[0] += 1
                    qa, qb = ((nc.sync, nc.gpsimd) if uidx[0] % 2
                              else (nc.gpsimd, nc.sync))
                    qa.dma_start(xx[0:RD, f0 + dn:f1 + dn],
                                 xx[RD:2 * RD, f0:f1])
                    qb.dma_start(xx[2 * RD:3 * RD, f0 - dn:f1 - dn],
                                 xx[RD:2 * RD, f0:f1])

                for l in range(L):
                    G, j = l // 4, l % 4
                    for grp in PAIRS_L[l]:
                        while len(pending) >= 3:
                            emit_back(*pending.pop(0))
                        # gated matmuls for all units of the pair first
                        pgfs = []
                        for (ss, ns, off) in grp:
                            a = ss % 4
                            pgt = pg.tile([128, 512], f32, tag="pgt",
                                          name="pgt")
                            pgs = pg.tile([128, 512], f32, tag="pgs",
                                          name="pgs")
                            pgfs.append((pgt, pgs))
                            for k in range(ns):
                                r = a + k
                                F = PAD + 512 * (ss + k)
                                for f, pgf in ((0, pgt), (1, pgs)):
                                    nc.tensor.matmul(
                                        pgf[32 * r:32 * (r + 1), :],
                                        wgk[0:96, (l * 2 + f) * 32:
                                            (l * 2 + f + 1) * 32],
                                        xx[0:96, F:F + 512],
                                        start=True, stop=True,
                                        tile_position=(0, 32 * r))
                        gt = tu.tile([128, 1024], bf16, tag="gt", name="gt")
                        gs = tu.tile([128, 1024], bf16, tag="gs", name="gs")
                        gm = tu.tile([128, 1024], bf16, tag="gm", name="gm")
                        p1m = 0
                        for (ss, ns, off), (pgt, pgs) in zip(grp, pgfs):
                            a = ss % 4
                            p1 = 32 * (a + ns)
                            p1m = max(p1m, p1)
                            nc.scalar.activation(
                                gt[0:p1, off * 512:(off + 1) * 512],
                                pgt[0:p1, :], AF.Tanh,
                                bias=btt[0:p1, l:l + 1])
                            nc.scalar.activation(
                                gs[0:p1, off * 512:(off + 1) * 512],
                                pgs[0:p1, :], AF.Sigmoid,
                                bias=bst[0:p1, l:l + 1])
                        cw = 512 * len(grp)
                        nc.vector.tensor_mul(gm[0:p1m, 0:cw], gt[0:p1m, 0:cw],
                                             gs[0:p1m, 0:cw])
                        # ring copies (permuted layout -> contiguous per row)
                        for r in range(4):
                            vs = []   # (off, sigma) valid strips in row r
                            for (ss, ns, off) in grp:
                                a = ss % 4
                                for k in range(ns):
                                    sig = ss + k
                                    if a + k == r and SV0 <= sig <= SV1:
                                        vs.append((off, sig))
                            if not vs:
                                continue
                            rp = _ring_pos(vs[0][1])
                            nv = len(vs)
                            nc.sync.dma_start(
                                rings[G][32 * j:32 * (j + 1),
                                         rp * 512:(rp + nv) * 512],
                                gm[32 * r:32 * (r + 1),
                                   vs[0][0] * 512:(vs[0][0] + nv) * 512])
                        if l < L - 1:
                            for (ss, ns, off) in grp:
                                pending.append((l, ss, ns, gm, off))
                while pending:
                    emit_back(*pending.pop(0))

            # ---------------- skip + post sweep ----------------
            with ExitStack() as pctx:
                sp = pctx.enter_context(tc.tile_pool(name=f"sp{half}", bufs=2))
                rrp = pctx.enter_context(tc.tile_pool(name=f"rr{half}", bufs=6))
                ch = pctx.enter_context(tc.tile_pool(name=f"ch{half}", bufs=1))
                psk = pctx.enter_context(
                    tc.tile_pool(name=f"psk{half}", bufs=5, space="PSUM"))
                pps = pctx.enter_context(
                    tc.tile_pool(name=f"pps{half}", bufs=1, space="PSUM"))
                psb = pctx.enter_context(
                    tc.tile_pool(name=f"psb{half}", bufs=1, space="PSUM"))

                rr_t = {}

                def emit_skip_group(cbg):
                    # 4 cbs; per m accumulate 7 ring matmuls per cb,
                    # cb-innermost for LDWEIGHTS reuse
                    for cb4 in range(4):
                        cb = cbg * 4 + cb4
                        rr_t[cb] = rrp.tile([128, 2048], bf16, tag="rr",
                                            name="rr")
                    for m in range(4):
                        pst = {}
                        for cb4 in range(4):
                            pst[cb4] = psk.tile([128, 512], f32, tag="sk",
                                                name="sk")
                        for g in range(NGRP):
                            nl = min(L - 4 * g, 4)      # layers in group
                            k1 = 32 * nl
                            for cb4 in range(4):
                                cb = cbg * 4 + cb4
                                rp = _ring_pos(SV0 + cb)
                                nc.tensor.matmul(
                                    pst[cb4][:],
                                    wskp[0:k1, (g * 4 + m) * 128:
                                         (g * 4 + m + 1) * 128],
                                    rings[g][0:k1, rp * 512:(rp + 1) * 512],
                                    start=(g == 0), stop=(g == NGRP - 1))
                        for cb4 in range(4):
                            cb = cbg * 4 + cb4
                            nc.scalar.activation(
                                rr_t[cb][:, m * 512:(m + 1) * 512],
                                pst[cb4][:], AF.Relu, bias=hb[:, m:m + 1])

                def emit_post1(cbg, oo8, sumball):
                    for cb4 in range(4):
                        cb = cbg * 4 + cb4
                        c8 = cb % 8
                        rr = rr_t.pop(cb)
                        pos = pps.tile([128, 1024], f32, tag="pos", name="pos")
                        for p in range(2):
                            for q in range(4):
                                nc.tensor.matmul(
                                    pos[:, p * 512:(p + 1) * 512],
                                    wp2[:, (q * 2 + p) * 128:
                                        (q * 2 + p + 1) * 128],
                                    rr[:, q * 512:(q + 1) * 512],
                                    start=(q == 0), stop=(q == 3))
                        ee = sp.tile([128, 1024], bf16, tag="ee", name="ee")
                        for p in range(2):
                            nc.scalar.activation(
                                ee[:, p * 512:(p + 1) * 512],
                                pos[:, p * 512:(p + 1) * 512],
                                AF.Exp, bias=bp2c[:, p:p + 1])
                            nc.scalar.activation(
                                oo8[:, c8 * 1024 + p * 512:
                                    c8 * 1024 + (p + 1) * 512],
                                pos[:, p * 512:(p + 1) * 512],
                                AF.Identity, bias=bp2c[:, p:p + 1])
                        psbt = psb.tile([128, 512], f32, tag="sb", name="sb")
                        for p in range(2):
                            nc.tensor.matmul(psbt[:], ones[:],
                                             ee[:, p * 512:(p + 1) * 512],
                                             start=(p == 0), stop=(p == 1))
                        nc.vector.tensor_copy(
                            sumball[:, c8 * 512:(c8 + 1) * 512], psbt[:])

                def emit_ln_post2(chunk, oo8, sumball):
                    lnb = ch.tile([128, 4096], bf16, tag="lnb", name="lnb")
                    nc.scalar.activation(lnb[:], sumball[:], AF.Ln)
                    for c8 in range(8):
                        cb = chunk * 8 + c8
                        oo2 = sp.tile([128, 1024], bf16, tag="oo2", name="oo2")
                        for p in range(2):
                            nc.vector.tensor_sub(
                                oo2[:, p * 512:(p + 1) * 512],
                                oo8[:, c8 * 1024 + p * 512:
                                    c8 * 1024 + (p + 1) * 512],
                                lnb[:, c8 * 512:(c8 + 1) * 512])
                            c0 = half * VH + cb * 512
                            nc.sync.dma_start(
                                out_d[p * 128:(p + 1) * 128, c0:c0 + 512],
                                oo2[:, p * 512:(p + 1) * 512])

                oo8s = [ch.tile([128, 8192], bf16, tag="oo8", name="oo8")
                        for _ in range(2)]
                sbs = [ch.tile([128, 4096], bf16, tag="sba", name="sba")
                       for _ in range(2)]
                for cbg in range(4):
                    emit_skip_group(cbg)
                    if cbg >= 1:
                        emit_post1(cbg - 1, oo8s[(cbg - 1) // 2],
                                   sbs[(cbg - 1) // 2])
                    if cbg == 2:
                        emit_ln_post2(0, oo8s[0], sbs[0])
                emit_post1(3, oo8s[1], sbs[1])
                emit_ln_post2(1, oo8s[1], sbs[1])

    nc.compile()
    return nc


def _prep_host(inputs):
    """Host-side exact fp32 preprocessing: initial conv, weight packing."""
    x = np.asarray(inputs["x"], np.float32)
    Wc = np.asarray(inputs["Wc"], np.float32)
    bc = np.asarray(inputs["bc"], np.float32)
    Wt = np.asarray(inputs["Wt"], np.float32)
    bt = np.asarray(inputs["bt"], np.float32)
    Ws = np.asarray(inputs["Ws"], np.float32)
    bs = np.asarray(inputs["bs"], np.float32)
    Wskip = np.asarray(inputs["Wskip"], np.float32)
    bskip = np.asarray(inputs["bskip"], np.float32)
    Wdense = np.asarray(inputs["Wdense"], np.float32)
    bdense = np.asarray(inputs["bdense"], np.float32)
    Wp1 = np.asarray(inputs["Wp1"], np.float32)
    bp1 = np.asarray(inputs["bp1"], np.float32)
    Wp2 = np.asarray(inputs["Wp2"], np.float32)
    bp2 = np.asarray(inputs["bp2"], np.float32)

    # initial conv (1 -> 32, k=3, pad=1), exact fp32 on host
    x0 = x[0, 0]
    xp = np.pad(x0, (1, 1))
    x1 = (Wc[:, 0, 0:1] * xp[None, 0:T]
          + Wc[:, 0, 1:2] * xp[None, 1:T + 1]
          + Wc[:, 0, 2:3] * xp[None, 2:T + 2]) + bc[:, None]
    xg = np.pad(x1, ((0, 0), (VOFF, VOFF)))

    # layer-0 tap replicas: row strip 0:32 holds x>>d0, 64:96 holds x<<d0
    d0 = DIL[0]
    xin = np.zeros((NCORES, 96, 2 * WA), BF16)
    for c in range(NCORES):
        for hf in range(2):
            s = c * V + hf * VH
            w = xg[:, s:s + WA].astype(BF16)
            o = hf * WA
            xin[c, RD:2 * RD, o:o + WA] = w
            xin[c, 0:RD, o + d0:o + WA] = w[:, :WA - d0]
            xin[c, 2 * RD:3 * RD, o:o + WA - d0] = w[:, d0:]

    wgk = np.zeros((96, L * 2 * 32), np.float32)
    for l in range(L):
        for f, W in ((0, Wt), (1, Ws)):
            for k in range(3):
                wgk[32 * k:32 * (k + 1),
                    (l * 2 + f) * 32:(l * 2 + f + 1) * 32] = W[l, :, :, k].T

    # block-diagonal stacked dense lhsT: rows 32s+j, cols 32s+k = Wdense[l,k,j]
    wdstk = np.zeros((128, L * 128), np.float32)
    for l in range(L):
        for s in range(4):
            wdstk[32 * s:32 * (s + 1),
                  l * 128 + 32 * s:l * 128 + 32 * (s + 1)] = \
                Wdense[l, :, :, 0].T

    W1s = np.einsum("ab,lbc->lac", Wp1[:, :, 0], Wskip[:, :, :, 0])  # [L,512,32]
    wskp = np.zeros((128, NGRP * 4 * 128), np.float32)
    for G in range(NGRP):
        for m in range(4):
            for jj in range(4):
                l = G * 4 + jj
                if l < L:
                    wskp[32 * jj:32 * (jj + 1),
                         (G * 4 + m) * 128:(G * 4 + m + 1) * 128] = \
                        W1s[l, 128 * m:128 * (m + 1), :].T

    wp2 = np.zeros((128, 8 * 128), np.float32)
    for q in range(4):
        for p in range(2):
            wp2[:, (q * 2 + p) * 128:(q * 2 + p + 1) * 128] = \
                Wp2[128 * p:128 * (p + 1), 128 * q:128 * (q + 1), 0].T

    hbias = Wp1[:, :, 0] @ bskip.sum(axis=0) + bp1     # [512]
    hb = hbias.reshape(4, 128).T.copy()                # [128, 4]

    shared = {
        "wgk": wgk.astype(BF16),
        "wdstk": wdstk.astype(BF16),
        "wskp": wskp.astype(BF16),
        "wp2": wp2.astype(BF16),
        "ones": np.ones((128, 128), BF16),
        "btt": np.ascontiguousarray(np.tile(bt.T, (4, 1)).astype(np.float32)),
        "bst": np.ascontiguousarray(np.tile(bs.T, (4, 1)).astype(np.float32)),
        "bdc": np.ascontiguousarray(np.tile(bdense.T, (2, 1)).astype(np.float32)),
        "hb": np.ascontiguousarray(hb.astype(np.float32)),
        "bp2c": np.ascontiguousarray(bp2.reshape(2, 128).T.astype(np.float32)),
    }
    return xin, shared


def kernel(**inputs):
    from concourse.bass_utils import run_bass_kernel_spmd

    xin, shared = _prep_host(inputs)
    if "nc" not in _cache:
        _cache["nc"] = _build()
    nc = _cache["nc"]

    in_maps = [dict(shared, xin=np.ascontiguousarray(xin[c]))
               for c in range(NCORES)]
    res = run_bass_kernel_spmd(nc, in_maps, core_ids=list(range(NCORES)))

    _last_run["nc"] = nc
    _last_run["in_maps"] = in_maps

    out = np.empty((1, QD, T), np.float32)
    for c in range(NCORES):
        out[0, :, c * V:(c + 1) * V] = res.results[c]["out"].astype(np.float32)
    return out


# revision 21
# speedup vs baseline: 1.2928x; 1.0538x over previous
"""Trainium2 Bass kernel for a WaveNet-style dilated-conv stack (v4).

Network (per reference):
  x1 = conv1d(x, Wc, bc, d=1, pad=1)                      # 1 -> 32, host-side
  for l in 27 layers, d = 2^(l%9):
      g = tanh(conv(x, Wt_l, d)) * sigmoid(conv(x, Ws_l, d))   # 32->32, k=3, pad=d
      skip += conv1x1(g, Wskip_l)                              # 32->512
      x = conv1x1(g, Wdense_l) + x
  out = conv1x1(relu(conv1x1(skip, Wp1)), Wp2)            # 512->512->256
  return log_softmax(out, axis=channels)

v4 design (8 cores, sequence-parallel, no cross-core comm):
  - Wp1 folded into skip weights (W1s = Wp1 @ Wskip) on host.
  - Per core 2 halves of 8192 cols; per-layer SHRINKING windows: layer l
    computes radius R_l = 512*ceil(S_{l+1}/512) (S = suffix dilation sum),
    R = 1536/1024/512/0.  Strip grid anchored at PAD; units are 4-strip
    aligned (8-strip super units + small boundary units).
  - Gated convs: one K=96 matmul per (strip, fn), col-tiled 4-way by
    sigma%4 into [128, 1024] f32 psum per fn; tanh/sigmoid at N=1024.
  - g stored stacked [128, 1024] bf16; ring copies to 7 per-group ring
    tiles [128, 8192] in a PERMUTED valid-strip order so each copy is one
    contiguous [32, <=1024] 4x-mode DVE op.  Rings for all 27 layers live
    simultaneously -> NO h accumulator, single skip sweep at end of half.
  - Dense conv: ONE stacked matmul (block-diag lhsT, M=128, K=32*strips)
    per 2048-block; evac+residual per strip: rows 1,3 -> DVE stt (psum
    cross-partition ok), rows 0,2 -> ScalarE Identity+bias to a [64,512]
    tmp at rows 32:64 + GpSimd tensor_add into xx (all same-partition).
  - Tap replicas (x>>d, x<<d on partition strips 0:32 / 64:96) maintained
    by 2 SBUF->SBUF DMAs per unit on the Sync queue.
  - Skip+post sweep per half: per 4-cb group, per m: 7 K-chunk matmuls
    (K=96 for the last 3-layer group: avoids reading unwritten ring rows),
    cb-inner for LDWEIGHTS reuse; DVE relu(x+hb) evac to rr.  Post per cb:
    8 Wp2 matmuls -> pos [128,1024] psum, Exp/Identity acts (bias bp2),
    sum-broadcast via 2 accumulating ones-matmuls, chunked Ln over 8 cbs
    (exp/ln stay in one table set per chunk -> no ACT_TABLE thrash), final
    log_softmax subtract on DVE, bf16 output DMA (host casts to f32).
"""

import numpy as np
import ml_dtypes

BF16 = ml_dtypes.bfloat16

DIL = [2 ** i for i in range(9)] * 3
L = len(DIL)            # 27
RD, SD, QD = 32, 512, 256
T = 131072
NCORES = 8
V = T // NCORES         # 16384 per core
VH = V // 2             # 8192 per half
HALO = 1536
PAD = 256
WH = VH + 2 * HALO      # 11264 max computed window per half
WA = WH + 2 * PAD       # 11776 allocated width per half
VOFF = HALO + PAD       # 1792 valid-region offset
NB = VH // 512          # 16 valid 512-col strips per half
NGRP = (L + 3) // 4     # 7 groups of (up to) 4 layers

# per-layer computed radius (cols), 512-aligned; S_{l+1} = sum(DIL[l+1:])
_S = [sum(DIL[i + 1:]) for i in range(L)]
R_L = [512 * ((s + 511) // 512) for s in _S]      # 1536x8, 1024x9, 512x9, 0
for _l in range(L):
    assert R_L[_l] >= _S[_l] and R_L[_l] <= HALO

# valid strips: global strip sigma covers flat cols [PAD+512s, PAD+512s+512)
SV0 = (VOFF - PAD) // 512        # 3 = first valid strip
SV1 = SV0 + NB - 1               # 18 = last valid strip


def _units_for(R):
    """Unit list [(sigma_start, n_strips)] for radius R; interior units are
    4-strip aligned, boundary-left unit stays within one 4-strip block."""
    s0 = SV0 - R // 512
    s1 = SV1 + 1 + R // 512
    units = []
    sa = s0
    if s0 % 4:
        ns = min(4 - s0 % 4, s1 - s0)
        units.append((s0, ns))
        sa = s0 + ns
    while sa < s1:
        ns = min(4, s1 - sa)
        units.append((sa, ns))
        sa += ns
    return units


def _pair_units(units):
    """Group consecutive 4-aligned units into pairs sharing one gm tile."""
    groups, i = [], 0
    while i < len(units):
        ss, ns = units[i]
        if (ss % 4 == 0 and i + 1 < len(units)
                and units[i + 1][0] % 4 == 0):
            groups.append([(ss, ns, 0), (units[i + 1][0], units[i + 1][1], 1)])
            i += 2
        else:
            groups.append([(ss, ns, 0)])
            i += 1
    return groups


UNITS_L = [_units_for(R) for R in R_L]
PAIRS_L = [_pair_units(u) for u in UNITS_L]


def _ring_pos(sigma):
    """Permuted ring column position for valid strip sigma (see docstring)."""
    r = sigma % 4
    b = sigma // 4
    bmin = 1 if r < 3 else 0
    return r * 4 + (b - bmin)


_cache = {}
_last_run = {}


def _build():
    from contextlib import ExitStack

    import concourse.bacc as bacc
    import concourse.mybir as mybir
    import concourse.tile as tile

    dt = mybir.dt
    AF = mybir.ActivationFunctionType
    ALU = mybir.AluOpType
    f32, bf16 = dt.float32, dt.bfloat16

    nc = bacc.Bacc("TRN2", target_bir_lowering=False, debug=False,
                   num_devices=NCORES)

    def din(name, shape, dty):
        return nc.dram_tensor(name, shape, dty, kind="ExternalInput").ap()

    xin_d = din("xin", [96, 2 * WA], bf16)
    wgk_d = din("wgk", [96, L * 2 * 32], bf16)    # 3-tap lhsT per (l, fn)
    wdstk_d = din("wdstk", [128, L * 128], bf16)  # block-diag dense lhsT
    wskp_d = din("wskp", [128, NGRP * 4 * 128], bf16)  # skip lhsT per (grp, m)
    wp2_d = din("wp2", [128, 8 * 128], bf16)      # Wp2 lhsT per (q, p)
    ones_d = din("ones", [128, 128], bf16)        # sum-broadcast lhsT
    btt_d = din("btt", [128, L], f32)             # bt stacked x4
    bst_d = din("bst", [128, L], f32)             # bs stacked x4
    bdcs_d = din("bdcs", [128, L], f32)           # bdense stacked x4
    hb_d = din("hb", [128, 4], f32)               # h bias per m
    bp2c_d = din("bp2c", [128, 2], f32)           # bp2 per p
    out_d = nc.dram_tensor("out", [QD, V], bf16, kind="ExternalOutput").ap()

    with tile.TileContext(nc) as tc, ExitStack() as top:
        wp = top.enter_context(tc.tile_pool(name="wp", bufs=1))

        def load(d, tag):
            t = wp.tile(list(d.shape), d.dtype, tag=tag, name=tag)
            nc.sync.dma_start(t[:], d[:])
            return t

        wgk = load(wgk_d, "wgk")
        wdstk = load(wdstk_d, "wdstk")
        wskp = load(wskp_d, "wskp")
        wp2 = load(wp2_d, "wp2")
        ones = load(ones_d, "ones")
        btt = load(btt_d, "btt")
        bst = load(bst_d, "bst")
        bdcs = load(bdcs_d, "bdcs")
        hb = load(hb_d, "hb")
        bp2c = load(bp2c_d, "bp2c")

        rings = [wp.tile([128, NB * 512], bf16, tag=f"ring{g}", name=f"ring{g}")
                 for g in range(NGRP)]

        for half in range(2):
            # ---------------- layer phase ----------------
            with ExitStack() as lctx:
                lp = lctx.enter_context(tc.tile_pool(name=f"lp{half}", bufs=1))
                pg = lctx.enter_context(
                    tc.tile_pool(name=f"pg{half}", bufs=2, space="PSUM"))
                pd = lctx.enter_context(
                    tc.tile_pool(name=f"pd{half}", bufs=3, space="PSUM"))
                tu = lctx.enter_context(tc.tile_pool(name=f"tu{half}", bufs=2))

                xx = lp.tile([96, WA], bf16, tag="xx", name="xx")
                nc.sync.dma_start(xx[:], xin_d[:, half * WA:(half + 1) * WA])

                # clear the rotating gated psum banks once per half so
                # unwritten rows read as finite values, never NaN
                for _ in range(2):
                    pgt0 = pg.tile([128, 512], f32, tag="pgt", name="pgt")
                    pgs0 = pg.tile([128, 512], f32, tag="pgs", name="pgs")
                    nc.vector.memset(pgt0[:], 0.0)
                    nc.vector.memset(pgs0[:], 0.0)

                pending = []
                uidx = [0]

                def emit_back(l2, ss2, ns2, gm2, off2):
                    # dense conv + x update + tap-replica DMAs for one unit
                    a2 = ss2 % 4
                    lw = wdstk[:, l2 * 128:(l2 + 1) * 128]
                    k1 = 32 * (a2 + ns2)
                    pdt = pd.tile([128, 512], f32, tag="pd", name="pd")
                    nc.tensor.matmul(
                        pdt[0:k1, :], lw[0:k1, 0:k1],
                        gm2[0:k1, off2 * 512:(off2 + 1) * 512],
                        start=True, stop=True)
                    # per-strip evac+residual: PSUM operand is exempt from
                    # the SBUF equal-base-partition rule, so the DVE stt can
                    # read psum rows 32r into xx rows 32:64.  One strip per
                    # unit goes through ScalarE+GpSimd to balance engines.
                    for kk in range(ns2):
                        r2 = a2 + kk
                        F = PAD + 512 * (ss2 + kk)
                        if r2 != 0:
                            nc.vector.scalar_tensor_tensor(
                                xx[RD:2 * RD, F:F + 512],
                                pdt[32 * r2:32 * (r2 + 1), :],
                                bdcs[RD:2 * RD, l2:l2 + 1],
                                xx[RD:2 * RD, F:F + 512],
                                op0=ALU.add, op1=ALU.add)
                        else:
                            tmp = tu.tile([64, 512], bf16, tag="tmp",
                                          name="tmp")
                            nc.scalar.activation(
                                tmp[RD:2 * RD, :],
                                pdt[32 * r2:32 * (r2 + 1), :],
                                AF.Identity, bias=bdcs[RD:2 * RD, l2:l2 + 1])
                            nc.gpsimd.tensor_add(
                                xx[RD:2 * RD, F:F + 512],
                                tmp[RD:2 * RD, :],
                                xx[RD:2 * RD, F:F + 512])
                    dn = DIL[l2 + 1]
                    f0 = PAD + 512 * ss2
                    f1 = PAD + 512 * (ss2 + ns2)
                    uidx[0] += 1
                    qa, qb = ((nc.sync, nc.gpsimd) if uidx[0] % 2
                              else (nc.gpsimd, nc.sync))
                    qa.dma_start(xx[0:RD, f0 + dn:f1 + dn],
                                 xx[RD:2 * RD, f0:f1])
                    qb.dma_start(xx[2 * RD:3 * RD, f0 - dn:f1 - dn],
                                 xx[RD:2 * RD, f0:f1])

                for l in range(L):
                    G, j = l // 4, l % 4
                    for grp in PAIRS_L[l]:
                        while len(pending) >= 3:
                            emit_back(*pending.pop(0))
                        # gated matmuls for all units of the pair first
                        pgfs = []
                        for (ss, ns, off) in grp:
                            a = ss % 4
                            pgt = pg.tile([128, 512], f32, tag="pgt",
                                          name="pgt")
                            pgs = pg.tile([128, 512], f32, tag="pgs",
                                          name="pgs")
                            pgfs.append((pgt, pgs))
                            for k in range(ns):
                                r = a + k
                                F = PAD + 512 * (ss + k)
                                for f, pgf in ((0, pgt), (1, pgs)):
                                    nc.tensor.matmul(
                                        pgf[32 * r:32 * (r + 1), :],
                                        wgk[0:96, (l * 2 + f) * 32:
                                            (l * 2 + f + 1) * 32],
                                        xx[0:96, F:F + 512],
                                        start=True, stop=True,
                                        tile_position=(0, 32 * r))
                        gt = tu.tile([128, 1024], bf16, tag="gt", name="gt")
                        gs = tu.tile([128, 1024], bf16, tag="gs", name="gs")
                        gm = tu.tile([128, 1024], bf16, tag="gm", name="gm")
                        p1m = 0
                        for (ss, ns, off), (pgt, pgs) in zip(grp, pgfs):
                            a = ss % 4
                            p1 = 32 * (a + ns)
                            p1m = max(p1m, p1)
                            nc.scalar.activation(
                                gt[0:p1, off * 512:(off + 1) * 512],
                                pgt[0:p1, :], AF.Tanh,
                                bias=btt[0:p1, l:l + 1])
                            nc.scalar.activation(
                                gs[0:p1, off * 512:(off + 1) * 512],
                                pgs[0:p1, :], AF.Sigmoid,
                                bias=bst[0:p1, l:l + 1])
                        cw = 512 * len(grp)
                        nc.vector.tensor_mul(gm[0:p1m, 0:cw], gt[0:p1m, 0:cw],
                                             gs[0:p1m, 0:cw])
                        # ring copies (permuted layout -> contiguous per row)
                        for r in range(4):
                            vs = []   # (off, sigma) valid strips in row r
                            for (ss, ns, off) in grp:
                                a = ss % 4
                                for k in range(ns):
                                    sig = ss + k
                                    if a + k == r and SV0 <= sig <= SV1:
                                        vs.append((off, sig))
                            if not vs:
                                continue
                            rp = _ring_pos(vs[0][1])
                            nv = len(vs)
                            rq = nc.gpsimd if (l + r) % 2 else nc.sync
                            rq.dma_start(
                                rings[G][32 * j:32 * (j + 1),
                                         rp * 512:(rp + nv) * 512],
                                gm[32 * r:32 * (r + 1),
                                   vs[0][0] * 512:(vs[0][0] + nv) * 512])
                        if l < L - 1:
                            for (ss, ns, off) in grp:
                                pending.append((l, ss, ns, gm, off))
                while pending:
                    emit_back(*pending.pop(0))

            # ---------------- skip + post sweep ----------------
            with ExitStack() as pctx:
                sp = pctx.enter_context(tc.tile_pool(name=f"sp{half}", bufs=2))
                rrp = pctx.enter_context(tc.tile_pool(name=f"rr{half}", bufs=6))
                ch = pctx.enter_context(tc.tile_pool(name=f"ch{half}", bufs=1))
                psk = pctx.enter_context(
                    tc.tile_pool(name=f"psk{half}", bufs=5, space="PSUM"))
                pps = pctx.enter_context(
                    tc.tile_pool(name=f"pps{half}", bufs=1, space="PSUM"))
                psb = pctx.enter_context(
                    tc.tile_pool(name=f"psb{half}", bufs=1, space="PSUM"))

                rr_t = {}

                def emit_skip_group(cbg):
                    # 4 cbs; per m accumulate 7 ring matmuls per cb,
                    # cb-innermost for LDWEIGHTS reuse
                    for cb4 in range(4):
                        cb = cbg * 4 + cb4
                        rr_t[cb] = rrp.tile([128, 2048], bf16, tag="rr",
                                            name="rr")
                    for m in range(4):
                        pst = {}
                        for cb4 in range(4):
                            pst[cb4] = psk.tile([128, 512], f32, tag="sk",
                                                name="sk")
                        for g in range(NGRP):
                            nl = min(L - 4 * g, 4)      # layers in group
                            k1 = 32 * nl
                            for cb4 in range(4):
                                cb = cbg * 4 + cb4
                                rp = _ring_pos(SV0 + cb)
                                nc.tensor.matmul(
                                    pst[cb4][:],
                                    wskp[0:k1, (g * 4 + m) * 128:
                                         (g * 4 + m + 1) * 128],
                                    rings[g][0:k1, rp * 512:(rp + 1) * 512],
                                    start=(g == 0), stop=(g == NGRP - 1))
                        for cb4 in range(4):
                            cb = cbg * 4 + cb4
                            nc.vector.tensor_scalar(
                                rr_t[cb][:, m * 512:(m + 1) * 512],
                                pst[cb4][:], hb[:, m:m + 1], 0.0,
                                op0=ALU.add, op1=ALU.max)

                def emit_post1(cbg, oo8, sumball):
                    for cb4 in range(4):
                        cb = cbg * 4 + cb4
                        c8 = cb % 8
                        rr = rr_t.pop(cb)
                        pos = pps.tile([128, 1024], f32, tag="pos", name="pos")
                        for p in range(2):
                            for q in range(4):
                                nc.tensor.matmul(
                                    pos[:, p * 512:(p + 1) * 512],
                                    wp2[:, (q * 2 + p) * 128:
                                        (q * 2 + p + 1) * 128],
                                    rr[:, q * 512:(q + 1) * 512],
                                    start=(q == 0), stop=(q == 3))
                        ee = sp.tile([128, 1024], bf16, tag="ee", name="ee")
                        for p in range(2):
                            nc.scalar.activation(
                                ee[:, p * 512:(p + 1) * 512],
                                pos[:, p * 512:(p + 1) * 512],
                                AF.Exp, bias=bp2c[:, p:p + 1])
                            nc.vector.tensor_scalar_add(
                                oo8[:, c8 * 1024 + p * 512:
                                    c8 * 1024 + (p + 1) * 512],
                                pos[:, p * 512:(p + 1) * 512],
                                bp2c[:, p:p + 1])
                        psbt = psb.tile([128, 512], f32, tag="sb", name="sb")
                        for p in range(2):
                            nc.tensor.matmul(psbt[:], ones[:],
                                             ee[:, p * 512:(p + 1) * 512],
                                             start=(p == 0), stop=(p == 1))
                        nc.vector.tensor_copy(
                            sumball[:, c8 * 512:(c8 + 1) * 512], psbt[:])

                def emit_ln_post2(chunk, oo8, sumball):
                    lnb = ch.tile([128, 4096], bf16, tag="lnb", name="lnb")
                    nc.scalar.activation(lnb[:], sumball[:], AF.Ln)
                    for c8 in range(8):
                        cb = chunk * 8 + c8
                        oo2 = sp.tile([128, 1024], bf16, tag="oo2", name="oo2")
                        for p in range(2):
                            nc.vector.tensor_sub(
                                oo2[:, p * 512:(p + 1) * 512],
                                oo8[:, c8 * 1024 + p * 512:
                                    c8 * 1024 + (p + 1) * 512],
                                lnb[:, c8 * 512:(c8 + 1) * 512])
                            c0 = half * VH + cb * 512
                            nc.sync.dma_start(
                                out_d[p * 128:(p + 1) * 128, c0:c0 + 512],
                                oo2[:, p * 512:(p + 1) * 512])

                oo8s = [ch.tile([128, 8192], bf16, tag="oo8", name="oo8")
                        for _ in range(2)]
                sbs = [ch.tile([128, 4096], bf16, tag="sba", name="sba")
                       for _ in range(2)]
                for cbg in range(4):
                    emit_skip_group(cbg)
                    if cbg >= 1:
                        emit_post1(cbg - 1, oo8s[(cbg - 1) // 2],
                                   sbs[(cbg - 1) // 2])
                    if cbg == 2:
                        emit_ln_post2(0, oo8s[0], sbs[0])
                emit_post1(3, oo8s[1], sbs[1])
                emit_ln_post2(1, oo8s[1], sbs[1])

    nc.compile()
    return nc


def _prep_host(inputs):
    """Host-side exact fp32 preprocessing: initial conv, weight packing."""
    x = np.asarray(inputs["x"], np.float32)
    Wc = np.asarray(inputs["Wc"], np.float32)
    bc = np.asarray(inputs["bc"], np.float32)
    Wt = np.asarray(inputs["Wt"], np.float32)
    bt = np.asarray(inputs["bt"], np.float32)
    Ws = np.asarray(inputs["Ws"], np.float32)
    bs = np.asarray(inputs["bs"], np.float32)
    Wskip = np.asarray(inputs["Wskip"], np.float32)
    bskip = np.asarray(inputs["bskip"], np.float32)
    Wdense = np.asarray(inputs["Wdense"], np.float32)
    bdense = np.asarray(inputs["bdense"], np.float32)
    Wp1 = np.asarray(inputs["Wp1"], np.float32)
    bp1 = np.asarray(inputs["bp1"], np.float32)
    Wp2 = np.asarray(inputs["Wp2"], np.float32)
    bp2 = np.asarray(inputs["bp2"], np.float32)

    # initial conv (1 -> 32, k=3, pad=1), exact fp32 on host
    x0 = x[0, 0]
    xp = np.pad(x0, (1, 1))
    x1 = (Wc[:, 0, 0:1] * xp[None, 0:T]
          + Wc[:, 0, 1:2] * xp[None, 1:T + 1]
          + Wc[:, 0, 2:3] * xp[None, 2:T + 2]) + bc[:, None]
    xg = np.pad(x1, ((0, 0), (VOFF, VOFF)))

    # layer-0 tap replicas: row strip 0:32 holds x>>d0, 64:96 holds x<<d0
    d0 = DIL[0]
    xin = np.zeros((NCORES, 96, 2 * WA), BF16)
    for c in range(NCORES):
        for hf in range(2):
            s = c * V + hf * VH
            w = xg[:, s:s + WA].astype(BF16)
            o = hf * WA
            xin[c, RD:2 * RD, o:o + WA] = w
            xin[c, 0:RD, o + d0:o + WA] = w[:, :WA - d0]
            xin[c, 2 * RD:3 * RD, o:o + WA - d0] = w[:, d0:]

    wgk = np.zeros((96, L * 2 * 32), np.float32)
    for l in range(L):
        for f, W in ((0, Wt), (1, Ws)):
            for k in range(3):
                wgk[32 * k:32 * (k + 1),
                    (l * 2 + f) * 32:(l * 2 + f + 1) * 32] = W[l, :, :, k].T

    # block-diagonal stacked dense lhsT: rows 32s+j, cols 32s+k = Wdense[l,k,j]
    wdstk = np.zeros((128, L * 128), np.float32)
    for l in range(L):
        for s in range(4):
            wdstk[32 * s:32 * (s + 1),
                  l * 128 + 32 * s:l * 128 + 32 * (s + 1)] = \
                Wdense[l, :, :, 0].T

    W1s = np.einsum("ab,lbc->lac", Wp1[:, :, 0], Wskip[:, :, :, 0])  # [L,512,32]
    wskp = np.zeros((128, NGRP * 4 * 128), np.float32)
    for G in range(NGRP):
        for m in range(4):
            for jj in range(4):
                l = G * 4 + jj
                if l < L:
                    wskp[32 * jj:32 * (jj + 1),
                         (G * 4 + m) * 128:(G * 4 + m + 1) * 128] = \
                        W1s[l, 128 * m:128 * (m + 1), :].T

    wp2 = np.zeros((128, 8 * 128), np.float32)
    for q in range(4):
        for p in range(2):
            wp2[:, (q * 2 + p) * 128:(q * 2 + p + 1) * 128] = \
                Wp2[128 * p:128 * (p + 1), 128 * q:128 * (q + 1), 0].T

    hbias = Wp1[:, :, 0] @ bskip.sum(axis=0) + bp1     # [512]
    hb = hbias.reshape(4, 128).T.copy()                # [128, 4]

    shared = {
        "wgk": wgk.astype(BF16),
        "wdstk": wdstk.astype(BF16),
        "wskp": wskp.astype(BF16),
        "wp2": wp2.astype(BF16),
        "ones": np.ones((128, 128), BF16),
        "btt": np.ascontiguousarray(np.tile(bt.T, (4, 1)).astype(np.float32)),
        "bst": np.ascontiguousarray(np.tile(bs.T, (4, 1)).astype(np.float32)),
        "bdcs": np.ascontiguousarray(np.tile(bdense.T, (4, 1)).astype(np.float32)),
        "hb": np.ascontiguousarray(hb.astype(np.float32)),
        "bp2c": np.ascontiguousarray(bp2.reshape(2, 128).T.astype(np.float32)),
    }
    return xin, shared


def kernel(**inputs):
    from concourse.bass_utils import run_bass_kernel_spmd

    xin, shared = _prep_host(inputs)
    if "nc" not in _cache:
        _cache["nc"] = _build()
    nc = _cache["nc"]

    in_maps = [dict(shared, xin=np.ascontiguousarray(xin[c]))
               for c in range(NCORES)]
    res = run_bass_kernel_spmd(nc, in_maps, core_ids=list(range(NCORES)))

    _last_run["nc"] = nc
    _last_run["in_maps"] = in_maps

    out = np.empty((1, QD, T), np.float32)
    for c in range(NCORES):
        out[0, :, c * V:(c + 1) * V] = res.results[c]["out"].astype(np.float32)
    return out


# revision 22
# speedup vs baseline: 1.2951x; 1.0018x over previous
"""Trainium2 Bass kernel for a WaveNet-style dilated-conv stack (v4).

Network (per reference):
  x1 = conv1d(x, Wc, bc, d=1, pad=1)                      # 1 -> 32, host-side
  for l in 27 layers, d = 2^(l%9):
      g = tanh(conv(x, Wt_l, d)) * sigmoid(conv(x, Ws_l, d))   # 32->32, k=3, pad=d
      skip += conv1x1(g, Wskip_l)                              # 32->512
      x = conv1x1(g, Wdense_l) + x
  out = conv1x1(relu(conv1x1(skip, Wp1)), Wp2)            # 512->512->256
  return log_softmax(out, axis=channels)

v4 design (8 cores, sequence-parallel, no cross-core comm):
  - Wp1 folded into skip weights (W1s = Wp1 @ Wskip) on host.
  - Per core 2 halves of 8192 cols; per-layer SHRINKING windows: layer l
    computes radius R_l = 512*ceil(S_{l+1}/512) (S = suffix dilation sum),
    R = 1536/1024/512/0.  Strip grid anchored at PAD; units are 4-strip
    aligned (8-strip super units + small boundary units).
  - Gated convs: one K=96 matmul per (strip, fn), col-tiled 4-way by
    sigma%4 into [128, 1024] f32 psum per fn; tanh/sigmoid at N=1024.
  - g stored stacked [128, 1024] bf16; ring copies to 7 per-group ring
    tiles [128, 8192] in a PERMUTED valid-strip order so each copy is one
    contiguous [32, <=1024] 4x-mode DVE op.  Rings for all 27 layers live
    simultaneously -> NO h accumulator, single skip sweep at end of half.
  - Dense conv: ONE stacked matmul (block-diag lhsT, M=128, K=32*strips)
    per 2048-block; evac+residual per strip: rows 1,3 -> DVE stt (psum
    cross-partition ok), rows 0,2 -> ScalarE Identity+bias to a [64,512]
    tmp at rows 32:64 + GpSimd tensor_add into xx (all same-partition).
  - Tap replicas (x>>d, x<<d on partition strips 0:32 / 64:96) maintained
    by 2 SBUF->SBUF DMAs per unit on the Sync queue.
  - Skip+post sweep per half: per 4-cb group, per m: 7 K-chunk matmuls
    (K=96 for the last 3-layer group: avoids reading unwritten ring rows),
    cb-inner for LDWEIGHTS reuse; DVE relu(x+hb) evac to rr.  Post per cb:
    8 Wp2 matmuls -> pos [128,1024] psum, Exp/Identity acts (bias bp2),
    sum-broadcast via 2 accumulating ones-matmuls, chunked Ln over 8 cbs
    (exp/ln stay in one table set per chunk -> no ACT_TABLE thrash), final
    log_softmax subtract on DVE, bf16 output DMA (host casts to f32).
"""

import numpy as np
import ml_dtypes

BF16 = ml_dtypes.bfloat16

DIL = [2 ** i for i in range(9)] * 3
L = len(DIL)            # 27
RD, SD, QD = 32, 512, 256
T = 131072
NCORES = 8
V = T // NCORES         # 16384 per core
VH = V // 2             # 8192 per half
HALO = 1536
PAD = 256
WH = VH + 2 * HALO      # 11264 max computed window per half
WA = WH + 2 * PAD       # 11776 allocated width per half
VOFF = HALO + PAD       # 1792 valid-region offset
NB = VH // 512          # 16 valid 512-col strips per half
NGRP = (L + 3) // 4     # 7 groups of (up to) 4 layers

# per-layer computed radius (cols), 512-aligned; S_{l+1} = sum(DIL[l+1:])
_S = [sum(DIL[i + 1:]) for i in range(L)]
R_L = [512 * ((s + 511) // 512) for s in _S]      # 1536x8, 1024x9, 512x9, 0
for _l in range(L):
    assert R_L[_l] >= _S[_l] and R_L[_l] <= HALO

# valid strips: global strip sigma covers flat cols [PAD+512s, PAD+512s+512)
SV0 = (VOFF - PAD) // 512        # 3 = first valid strip
SV1 = SV0 + NB - 1               # 18 = last valid strip


def _units_for(R):
    """Unit list [(sigma_start, n_strips)] for radius R; interior units are
    4-strip aligned, boundary-left unit stays within one 4-strip block."""
    s0 = SV0 - R // 512
    s1 = SV1 + 1 + R // 512
    units = []
    sa = s0
    if s0 % 4:
        ns = min(4 - s0 % 4, s1 - s0)
        units.append((s0, ns))
        sa = s0 + ns
    while sa < s1:
        ns = min(4, s1 - sa)
        units.append((sa, ns))
        sa += ns
    return units


def _pair_units(units):
    """Group consecutive 4-aligned units into pairs sharing one gm tile."""
    groups, i = [], 0
    while i < len(units):
        ss, ns = units[i]
        if (ss % 4 == 0 and i + 1 < len(units)
                and units[i + 1][0] % 4 == 0):
            groups.append([(ss, ns, 0), (units[i + 1][0], units[i + 1][1], 1)])
            i += 2
        else:
            groups.append([(ss, ns, 0)])
            i += 1
    return groups


UNITS_L = [_units_for(R) for R in R_L]
PAIRS_L = [_pair_units(u) for u in UNITS_L]


def _ring_pos(sigma):
    """Permuted ring column position for valid strip sigma (see docstring)."""
    r = sigma % 4
    b = sigma // 4
    bmin = 1 if r < 3 else 0
    return r * 4 + (b - bmin)


_cache = {}
_last_run = {}


def _build():
    from contextlib import ExitStack

    import concourse.bacc as bacc
    import concourse.mybir as mybir
    import concourse.tile as tile

    dt = mybir.dt
    AF = mybir.ActivationFunctionType
    ALU = mybir.AluOpType
    f32, bf16 = dt.float32, dt.bfloat16

    nc = bacc.Bacc("TRN2", target_bir_lowering=False, debug=False,
                   num_devices=NCORES)

    def din(name, shape, dty):
        return nc.dram_tensor(name, shape, dty, kind="ExternalInput").ap()

    xin_d = din("xin", [96, 2 * WA], bf16)
    wgk_d = din("wgk", [96, L * 2 * 32], bf16)    # 3-tap lhsT per (l, fn)
    wdstk_d = din("wdstk", [128, L * 128], bf16)  # block-diag dense lhsT
    wskp_d = din("wskp", [128, NGRP * 4 * 128], bf16)  # skip lhsT per (grp, m)
    wp2_d = din("wp2", [128, 8 * 128], bf16)      # Wp2 lhsT per (q, p)
    ones_d = din("ones", [128, 128], bf16)        # sum-broadcast lhsT
    btt_d = din("btt", [128, L], f32)             # bt stacked x4
    bst_d = din("bst", [128, L], f32)             # bs stacked x4
    bdcs_d = din("bdcs", [128, L], f32)           # bdense stacked x4
    hb_d = din("hb", [128, 4], f32)               # h bias per m
    bp2c_d = din("bp2c", [128, 2], f32)           # bp2 per p
    out_d = nc.dram_tensor("out", [QD, V], bf16, kind="ExternalOutput").ap()

    with tile.TileContext(nc) as tc, ExitStack() as top:
        wp = top.enter_context(tc.tile_pool(name="wp", bufs=1))

        def load(d, tag):
            t = wp.tile(list(d.shape), d.dtype, tag=tag, name=tag)
            nc.sync.dma_start(t[:], d[:])
            return t

        wgk = load(wgk_d, "wgk")
        wdstk = load(wdstk_d, "wdstk")
        wskp = load(wskp_d, "wskp")
        wp2 = load(wp2_d, "wp2")
        ones = load(ones_d, "ones")
        btt = load(btt_d, "btt")
        bst = load(bst_d, "bst")
        bdcs = load(bdcs_d, "bdcs")
        hb = load(hb_d, "hb")
        bp2c = load(bp2c_d, "bp2c")

        rings = [wp.tile([128, NB * 512], bf16, tag=f"ring{g}", name=f"ring{g}")
                 for g in range(NGRP)]

        for half in range(2):
            # ---------------- layer phase ----------------
            with ExitStack() as lctx:
                lp = lctx.enter_context(tc.tile_pool(name=f"lp{half}", bufs=1))
                pg = lctx.enter_context(
                    tc.tile_pool(name=f"pg{half}", bufs=2, space="PSUM"))
                pd = lctx.enter_context(
                    tc.tile_pool(name=f"pd{half}", bufs=3, space="PSUM"))
                tu = lctx.enter_context(tc.tile_pool(name=f"tu{half}", bufs=2))

                xx = lp.tile([96, WA], bf16, tag="xx", name="xx")
                nc.sync.dma_start(xx[:], xin_d[:, half * WA:(half + 1) * WA])

                # clear the rotating gated psum banks once per half so
                # unwritten rows read as finite values, never NaN
                for _ in range(2):
                    pgt0 = pg.tile([128, 512], f32, tag="pgt", name="pgt")
                    pgs0 = pg.tile([128, 512], f32, tag="pgs", name="pgs")
                    nc.vector.memset(pgt0[:], 0.0)
                    nc.vector.memset(pgs0[:], 0.0)

                pending = []
                uidx = [0]

                def emit_back(l2, ss2, ns2, gm2, off2):
                    # dense conv + x update + tap-replica DMAs for one unit
                    a2 = ss2 % 4
                    lw = wdstk[:, l2 * 128:(l2 + 1) * 128]
                    k1 = 32 * (a2 + ns2)
                    pdt = pd.tile([128, 512], f32, tag="pd", name="pd")
                    nc.tensor.matmul(
                        pdt[0:k1, :], lw[0:k1, 0:k1],
                        gm2[0:k1, off2 * 512:(off2 + 1) * 512],
                        start=True, stop=True)
                    # per-strip evac+residual: PSUM operand is exempt from
                    # the SBUF equal-base-partition rule, so the DVE stt can
                    # read psum rows 32r into xx rows 32:64.  One strip per
                    # unit goes through ScalarE+GpSimd to balance engines.
                    for kk in range(ns2):
                        r2 = a2 + kk
                        F = PAD + 512 * (ss2 + kk)
                        if r2 == 1 or r2 == 3 or (r2 == 2 and ss2 % 8 < 4):
                            nc.vector.scalar_tensor_tensor(
                                xx[RD:2 * RD, F:F + 512],
                                pdt[32 * r2:32 * (r2 + 1), :],
                                bdcs[RD:2 * RD, l2:l2 + 1],
                                xx[RD:2 * RD, F:F + 512],
                                op0=ALU.add, op1=ALU.add)
                        else:
                            tmp = tu.tile([64, 512], bf16, tag="tmp",
                                          name="tmp")
                            nc.scalar.activation(
                                tmp[RD:2 * RD, :],
                                pdt[32 * r2:32 * (r2 + 1), :],
                                AF.Identity, bias=bdcs[RD:2 * RD, l2:l2 + 1])
                            nc.gpsimd.tensor_add(
                                xx[RD:2 * RD, F:F + 512],
                                tmp[RD:2 * RD, :],
                                xx[RD:2 * RD, F:F + 512])
                    dn = DIL[l2 + 1]
                    f0 = PAD + 512 * ss2
                    f1 = PAD + 512 * (ss2 + ns2)
                    uidx[0] += 1
                    qa, qb = ((nc.sync, nc.gpsimd) if uidx[0] % 2
                              else (nc.gpsimd, nc.sync))
                    qa.dma_start(xx[0:RD, f0 + dn:f1 + dn],
                                 xx[RD:2 * RD, f0:f1])
                    qb.dma_start(xx[2 * RD:3 * RD, f0 - dn:f1 - dn],
                                 xx[RD:2 * RD, f0:f1])

                for l in range(L):
                    G, j = l // 4, l % 4
                    for grp in PAIRS_L[l]:
                        while len(pending) >= 3:
                            emit_back(*pending.pop(0))
                        # gated matmuls for all units of the pair first
                        pgfs = []
                        for (ss, ns, off) in grp:
                            a = ss % 4
                            pgt = pg.tile([128, 512], f32, tag="pgt",
                                          name="pgt")
                            pgs = pg.tile([128, 512], f32, tag="pgs",
                                          name="pgs")
                            pgfs.append((pgt, pgs))
                            for k in range(ns):
                                r = a + k
                                F = PAD + 512 * (ss + k)
                                for f, pgf in ((0, pgt), (1, pgs)):
                                    nc.tensor.matmul(
                                        pgf[32 * r:32 * (r + 1), :],
                                        wgk[0:96, (l * 2 + f) * 32:
                                            (l * 2 + f + 1) * 32],
                                        xx[0:96, F:F + 512],
                                        start=True, stop=True,
                                        tile_position=(0, 32 * r))
                        gt = tu.tile([128, 1024], bf16, tag="gt", name="gt")
                        gs = tu.tile([128, 1024], bf16, tag="gs", name="gs")
                        gm = tu.tile([128, 1024], bf16, tag="gm", name="gm")
                        p1m = 0
                        for (ss, ns, off), (pgt, pgs) in zip(grp, pgfs):
                            a = ss % 4
                            p1 = 32 * (a + ns)
                            p1m = max(p1m, p1)
                            nc.scalar.activation(
                                gt[0:p1, off * 512:(off + 1) * 512],
                                pgt[0:p1, :], AF.Tanh,
                                bias=btt[0:p1, l:l + 1])
                            nc.scalar.activation(
                                gs[0:p1, off * 512:(off + 1) * 512],
                                pgs[0:p1, :], AF.Sigmoid,
                                bias=bst[0:p1, l:l + 1])
                        cw = 512 * len(grp)
                        nc.vector.tensor_mul(gm[0:p1m, 0:cw], gt[0:p1m, 0:cw],
                                             gs[0:p1m, 0:cw])
                        # ring copies (permuted layout -> contiguous per row)
                        for r in range(4):
                            vs = []   # (off, sigma) valid strips in row r
                            for (ss, ns, off) in grp:
                                a = ss % 4
                                for k in range(ns):
                                    sig = ss + k
                                    if a + k == r and SV0 <= sig <= SV1:
                                        vs.append((off, sig))
                            if not vs:
                                continue
                            rp = _ring_pos(vs[0][1])
                            nv = len(vs)
                            nc.sync.dma_start(
                                rings[G][32 * j:32 * (j + 1),
                                         rp * 512:(rp + nv) * 512],
                                gm[32 * r:32 * (r + 1),
                                   vs[0][0] * 512:(vs[0][0] + nv) * 512])
                        if l < L - 1:
                            for (ss, ns, off) in grp:
                                pending.append((l, ss, ns, gm, off))
                while pending:
                    emit_back(*pending.pop(0))

            # ---------------- skip + post sweep ----------------
            with ExitStack() as pctx:
                sp = pctx.enter_context(tc.tile_pool(name=f"sp{half}", bufs=2))
                rrp = pctx.enter_context(tc.tile_pool(name=f"rr{half}", bufs=6))
                ch = pctx.enter_context(tc.tile_pool(name=f"ch{half}", bufs=1))
                psk = pctx.enter_context(
                    tc.tile_pool(name=f"psk{half}", bufs=5, space="PSUM"))
                pps = pctx.enter_context(
                    tc.tile_pool(name=f"pps{half}", bufs=1, space="PSUM"))
                psb = pctx.enter_context(
                    tc.tile_pool(name=f"psb{half}", bufs=1, space="PSUM"))

                rr_t = {}

                def emit_skip_group(cbg):
                    # 4 cbs; per m accumulate 7 ring matmuls per cb,
                    # cb-innermost for LDWEIGHTS reuse
                    for cb4 in range(4):
                        cb = cbg * 4 + cb4
                        rr_t[cb] = rrp.tile([128, 2048], bf16, tag="rr",
                                            name="rr")
                    for m in range(4):
                        pst = {}
                        for cb4 in range(4):
                            pst[cb4] = psk.tile([128, 512], f32, tag="sk",
                                                name="sk")
                        for g in range(NGRP):
                            nl = min(L - 4 * g, 4)      # layers in group
                            k1 = 32 * nl
                            for cb4 in range(4):
                                cb = cbg * 4 + cb4
                                rp = _ring_pos(SV0 + cb)
                                nc.tensor.matmul(
                                    pst[cb4][:],
                                    wskp[0:k1, (g * 4 + m) * 128:
                                         (g * 4 + m + 1) * 128],
                                    rings[g][0:k1, rp * 512:(rp + 1) * 512],
                                    start=(g == 0), stop=(g == NGRP - 1))
                        for cb4 in range(4):
                            cb = cbg * 4 + cb4
                            nc.vector.tensor_scalar(
                                rr_t[cb][:, m * 512:(m + 1) * 512],
                                pst[cb4][:], hb[:, m:m + 1], 0.0,
                                op0=ALU.add, op1=ALU.max)

                def emit_post1(cbg, oo8, sumball):
                    for cb4 in range(4):
                        cb = cbg * 4 + cb4
                        c8 = cb % 8
                        rr = rr_t.pop(cb)
                        pos = pps.tile([128, 1024], f32, tag="pos", name="pos")
                        for p in range(2):
                            for q in range(4):
                                nc.tensor.matmul(
                                    pos[:, p * 512:(p + 1) * 512],
                                    wp2[:, (q * 2 + p) * 128:
                                        (q * 2 + p + 1) * 128],
                                    rr[:, q * 512:(q + 1) * 512],
                                    start=(q == 0), stop=(q == 3))
                        ee = sp.tile([128, 1024], bf16, tag="ee", name="ee")
                        for p in range(2):
                            nc.scalar.activation(
                                ee[:, p * 512:(p + 1) * 512],
                                pos[:, p * 512:(p + 1) * 512],
                                AF.Exp, bias=bp2c[:, p:p + 1])
                            nc.vector.tensor_scalar_add(
                                oo8[:, c8 * 1024 + p * 512:
                                    c8 * 1024 + (p + 1) * 512],
                                pos[:, p * 512:(p + 1) * 512],
                                bp2c[:, p:p + 1])
                        psbt = psb.tile([128, 512], f32, tag="sb", name="sb")
                        for p in range(2):
                            nc.tensor.matmul(psbt[:], ones[:],
                                             ee[:, p * 512:(p + 1) * 512],
                                             start=(p == 0), stop=(p == 1))
                        nc.vector.tensor_copy(
                            sumball[:, c8 * 512:(c8 + 1) * 512], psbt[:])

                def emit_ln_post2(chunk, oo8, sumball):
                    lnb = ch.tile([128, 4096], bf16, tag="lnb", name="lnb")
                    nc.scalar.activation(lnb[:], sumball[:], AF.Ln)
                    for c8 in range(8):
                        cb = chunk * 8 + c8
                        oo2 = sp.tile([128, 1024], bf16, tag="oo2", name="oo2")
                        for p in range(2):
                            nc.vector.tensor_sub(
                                oo2[:, p * 512:(p + 1) * 512],
                                oo8[:, c8 * 1024 + p * 512:
                                    c8 * 1024 + (p + 1) * 512],
                                lnb[:, c8 * 512:(c8 + 1) * 512])
                            c0 = half * VH + cb * 512
                            nc.sync.dma_start(
                                out_d[p * 128:(p + 1) * 128, c0:c0 + 512],
                                oo2[:, p * 512:(p + 1) * 512])

                oo8s = [ch.tile([128, 8192], bf16, tag="oo8", name="oo8")
                        for _ in range(2)]
                sbs = [ch.tile([128, 4096], bf16, tag="sba", name="sba")
                       for _ in range(2)]
                for cbg in range(4):
                    emit_skip_group(cbg)
                    if cbg >= 1:
                        emit_post1(cbg - 1, oo8s[(cbg - 1) // 2],
                                   sbs[(cbg - 1) // 2])
                    if cbg == 2:
                        emit_ln_post2(0, oo8s[0], sbs[0])
                emit_post1(3, oo8s[1], sbs[1])
                emit_ln_post2(1, oo8s[1], sbs[1])

    nc.compile()
    return nc


def _prep_host(inputs):
    """Host-side exact fp32 preprocessing: initial conv, weight packing."""
    x = np.asarray(inputs["x"], np.float32)
    Wc = np.asarray(inputs["Wc"], np.float32)
    bc = np.asarray(inputs["bc"], np.float32)
    Wt = np.asarray(inputs["Wt"], np.float32)
    bt = np.asarray(inputs["bt"], np.float32)
    Ws = np.asarray(inputs["Ws"], np.float32)
    bs = np.asarray(inputs["bs"], np.float32)
    Wskip = np.asarray(inputs["Wskip"], np.float32)
    bskip = np.asarray(inputs["bskip"], np.float32)
    Wdense = np.asarray(inputs["Wdense"], np.float32)
    bdense = np.asarray(inputs["bdense"], np.float32)
    Wp1 = np.asarray(inputs["Wp1"], np.float32)
    bp1 = np.asarray(inputs["bp1"], np.float32)
    Wp2 = np.asarray(inputs["Wp2"], np.float32)
    bp2 = np.asarray(inputs["bp2"], np.float32)

    # initial conv (1 -> 32, k=3, pad=1), exact fp32 on host
    x0 = x[0, 0]
    xp = np.pad(x0, (1, 1))
    x1 = (Wc[:, 0, 0:1] * xp[None, 0:T]
          + Wc[:, 0, 1:2] * xp[None, 1:T + 1]
          + Wc[:, 0, 2:3] * xp[None, 2:T + 2]) + bc[:, None]
    xg = np.pad(x1, ((0, 0), (VOFF, VOFF)))

    # layer-0 tap replicas: row strip 0:32 holds x>>d0, 64:96 holds x<<d0
    d0 = DIL[0]
    xin = np.zeros((NCORES, 96, 2 * WA), BF16)
    for c in range(NCORES):
        for hf in range(2):
            s = c * V + hf * VH
            w = xg[:, s:s + WA].astype(BF16)
            o = hf * WA
            xin[c, RD:2 * RD, o:o + WA] = w
            xin[c, 0:RD, o + d0:o + WA] = w[:, :WA - d0]
            xin[c, 2 * RD:3 * RD, o:o + WA - d0] = w[:, d0:]

    wgk = np.zeros((96, L * 2 * 32), np.float32)
    for l in range(L):
        for f, W in ((0, Wt), (1, Ws)):
            for k in range(3):
                wgk[32 * k:32 * (k + 1),
                    (l * 2 + f) * 32:(l * 2 + f + 1) * 32] = W[l, :, :, k].T

    # block-diagonal stacked dense lhsT: rows 32s+j, cols 32s+k = Wdense[l,k,j]
    wdstk = np.zeros((128, L * 128), np.float32)
    for l in range(L):
        for s in range(4):
            wdstk[32 * s:32 * (s + 1),
                  l * 128 + 32 * s:l * 128 + 32 * (s + 1)] = \
                Wdense[l, :, :, 0].T

    W1s = np.einsum("ab,lbc->lac", Wp1[:, :, 0], Wskip[:, :, :, 0])  # [L,512,32]
    wskp = np.zeros((128, NGRP * 4 * 128), np.float32)
    for G in range(NGRP):
        for m in range(4):
            for jj in range(4):
                l = G * 4 + jj
                if l < L:
                    wskp[32 * jj:32 * (jj + 1),
                         (G * 4 + m) * 128:(G * 4 + m + 1) * 128] = \
                        W1s[l, 128 * m:128 * (m + 1), :].T

    wp2 = np.zeros((128, 8 * 128), np.float32)
    for q in range(4):
        for p in range(2):
            wp2[:, (q * 2 + p) * 128:(q * 2 + p + 1) * 128] = \
                Wp2[128 * p:128 * (p + 1), 128 * q:128 * (q + 1), 0].T

    hbias = Wp1[:, :, 0] @ bskip.sum(axis=0) + bp1     # [512]
    hb = hbias.reshape(4, 128).T.copy()                # [128, 4]

    shared = {
        "wgk": wgk.astype(BF16),
        "wdstk": wdstk.astype(BF16),
        "wskp": wskp.astype(BF16),
        "wp2": wp2.astype(BF16),
        "ones": np.ones((128, 128), BF16),
        "btt": np.ascontiguousarray(np.tile(bt.T, (4, 1)).astype(np.float32)),
        "bst": np.ascontiguousarray(np.tile(bs.T, (4, 1)).astype(np.float32)),
        "bdcs": np.ascontiguousarray(np.tile(bdense.T, (4, 1)).astype(np.float32)),
        "hb": np.ascontiguousarray(hb.astype(np.float32)),
        "bp2c": np.ascontiguousarray(bp2.reshape(2, 128).T.astype(np.float32)),
    }
    return xin, shared


def kernel(**inputs):
    from concourse.bass_utils import run_bass_kernel_spmd

    xin, shared = _prep_host(inputs)
    if "nc" not in _cache:
        _cache["nc"] = _build()
    nc = _cache["nc"]

    in_maps = [dict(shared, xin=np.ascontiguousarray(xin[c]))
               for c in range(NCORES)]
    res = run_bass_kernel_spmd(nc, in_maps, core_ids=list(range(NCORES)))

    _last_run["nc"] = nc
    _last_run["in_maps"] = in_maps

    out = np.empty((1, QD, T), np.float32)
    for c in range(NCORES):
        out[0, :, c * V:(c + 1) * V] = res.results[c]["out"].astype(np.float32)
    return out
